# revision 1
# baseline (speedup 1.0000x reference)
"""Trainium2 Bass kernel for nn_CSDC_8246337208509 (I_LCA block: CAB cross-attention + IEL gated FFN).

Contract: kernel(**inputs) takes FULL unsharded inputs, returns FULL output.
Sharding: 8 cores = 4 batches x 2 spatial halves (128 rows of H each).
Two device launches with a tiny host-side combine (attention softmax over 8x8
per-head Gram matrices) between them.
"""

import sys

import numpy as np

try:
    import concourse.bass as bass  # noqa: F401
except Exception:  # pragma: no cover
    sys.path.insert(0, "/opt/trn_rl_repo")
    sys.path.insert(0, "/root/.axon_site/_ro/trn_rl_repo")

import concourse.bacc as bacc
import concourse.tile as tile
from concourse import mybir
from concourse import bass_utils
from concourse.alu_op_type import AluOpType
import ml_dtypes

BF16 = ml_dtypes.bfloat16
F32 = np.float32
BT = mybir.dt.bfloat16
FT = mybir.dt.float32

B, C, H, W = 4, 64, 256, 256
HEADS, CH = 8, 8
HID = 170
EPS = 1e-6
Wp = W + 2  # 258, zero col at 0 and 257
HS = H // 2  # 128 interior rows per core
Hb1 = 16  # k1 band interior rows
NB1 = HS // Hb1
Hb2 = 8  # k2 band interior rows
NB2 = HS // Hb2
NCORES = 8

TAPS = [(ty - 1, tx - 1) for ty in range(3) for tx in range(3)]  # (dy, dx), t = ty*3+tx

# channel placement for the 340-wide IEL stream into 3 groups of 128 partitions:
# G0 = x1[0:128]; G1 = x2[0:128]; G2: slots 0..41 = x1[128:170], slots 64..105 =
# x2[128:170] (partition bases must be 32-aligned, so the x2 tail sits at 64).
PLACE340 = [-1] * 384
for _i in range(128):
    PLACE340[_i] = _i           # G0
    PLACE340[128 + _i] = 170 + _i  # G1
for _i in range(42):
    PLACE340[256 + _i] = 128 + _i  # G2 low: x1 tail
    PLACE340[256 + 64 + _i] = 298 + _i  # G2 high: x2 tail
NG = 3


# ---------------------------------------------------------------- device code

def _ln_into(nc, tc, pools, src, nrows, dst, affine, src_f32=False):
    """Channels-first LayerNorm of src[:, :nrows, 1:257] -> dst (S-stacked bf16).

    src: [64, nrows, 258] bf16 tile. dst: [128, nrows, 260] S-layout tile whose
    pad cols are already zeroed: top half dst[0:64, r, c] = ln[r, c-1] (written
    at cols 2:258), bottom half dst[64:128, r, c] = ln[r, c+1] (gpsimd copy).
    All transposes ride the DMA xbar (bf16), not the PE.
    """
    lnscr = pools["lnscr"]
    T = nrows * 2
    xTs = lnscr.tile([128, T, 64], BT, tag="ln_xTs")
    xnT = lnscr.tile([128, T, 64], BT, tag="ln_xnT")
    st = lnscr.tile([128, T, 6], FT, tag="ln_st")
    mv = lnscr.tile([128, T, 2], FT, tag="ln_mv")
    sr = lnscr.tile([128, T, 1], FT, tag="ln_sr")
    ri = lnscr.tile([128, T, 1], FT, tag="ln_ri")

    ps_t = pools["ps_t"]
    ident = pools["idf"] if src_f32 else pools["idb"]
    for g in range((T + 7) // 8):
        n = min(8, T - g * 8)
        pt = ps_t.tile([128, 8, 64], FT if src_f32 else BT, tag="ps_fw")
        for j in range(n):
            t = g * 8 + j
            row, half = t // 2, t % 2
            nc.tensor.transpose(
                pt[:, j, :],
                src[:, row, 1 + 128 * half : 1 + 128 * half + 128],
                ident[0:64, 0:64],
            )
        (nc.scalar.copy if g % 2 == 0 else nc.vector.tensor_copy)(
            xTs[:, g * 8 : g * 8 + n, :], pt[:, 0:n, :]
        )
    for t in range(T):
        nc.vector.bn_stats(st[:, t, :], xTs[:, t, :])
        nc.vector.bn_aggr(mv[:, t, :], st[:, t, :])
    nc.scalar.activation(sr, mv[:, :, 1:2], mybir.ActivationFunctionType.Sqrt, bias=pools["eps"])
    nc.vector.reciprocal(ri, sr)
    for t in range(T):
        nc.vector.tensor_scalar(
            out=xnT[:, t, :],
            in0=xTs[:, t, :],
            scalar1=mv[:, t, 0:1],
            scalar2=ri[:, t, 0:1],
            op0=AluOpType.subtract,
            op1=AluOpType.mult,
        )
    if affine:
        gam_bc, bet_bc = pools["gam_bc"], pools["bet_bc"]
        for t in range(T):
            nc.vector.tensor_tensor(out=xnT[:, t, :], in0=xnT[:, t, :], in1=gam_bc, op=AluOpType.mult)
            nc.vector.tensor_tensor(out=xnT[:, t, :], in0=xnT[:, t, :], in1=bet_bc, op=AluOpType.add)
    # transpose back (PE) into the S-layout top half, then gpsimd-fill the bottom
    for g in range((T + 3) // 4):
        pb = ps_t.tile([128, 2, 256], BT, tag="ps_bw")
        for j in range(4):
            t = g * 4 + j
            nc.tensor.transpose(
                pb[0:64, j // 2, 128 * (j % 2) : 128 * (j % 2) + 128],
                xnT[:, t, :],
                pools["idb"],
            )
        (nc.scalar.copy if g % 2 == 0 else nc.vector.tensor_copy)(
            dst[0:64, g * 2 : g * 2 + 2, 2:258], pb[0:64]
        )
        nc.gpsimd.tensor_copy(
            dst[64:128, g * 2 : g * 2 + 2, 0:256], dst[0:64, g * 2 : g * 2 + 2, 2:258]
        )


def _zero_pad_cols(nc, t, nrows):
    nc.gpsimd.memset(t[:, 0:nrows, 0:1], 0.0)
    nc.gpsimd.memset(t[:, 0:nrows, 257:258], 0.0)


def _zero_pad_cols_s(nc, t, nrows):
    # S-stacked layout [128, nrows, 260]: top half holds u[c-1], bottom u[c+1]
    nc.gpsimd.memset(t[0:64, 0:nrows, 0:2], 0.0)
    nc.gpsimd.memset(t[0:64, 0:nrows, 258:260], 0.0)
    nc.gpsimd.memset(t[64:128, 0:nrows, 256:260], 0.0)


# fused conv1x1+dw3x3: 3 K=128 pair-matmuls + 3 K=64 single-matmuls per chunk.
# S: [128, nr, 260] stacked input; out rows j correspond to S rows j+roff.
def _fused_conv(nc, ps_pool, pairs, sings, S, roff, nchunks, evict, M=128):
    for c in range(nchunks):
        pt = ps_pool.tile([128, 2, W], FT, tag="ps_mm")
        for p in range(3):
            dy = p - 1
            nc.tensor.matmul(
                pt[0:M],
                lhsT=pairs[:, p, :],
                rhs=S[:, roff + 2 * c + dy : roff + 2 * c + dy + 2, 1:257],
                start=(p == 0),
                stop=False,
            )
        for i in range(3):
            dy = i - 1
            nc.tensor.matmul(
                pt[0:M],
                lhsT=sings[:, i, :],
                rhs=S[0:64, roff + 2 * c + dy : roff + 2 * c + dy + 2, 2:258],
                start=False,
                stop=(i == 2),
            )
        evict(c, pt[0:M])


def _build_k1(affine):
    nc = bacc.Bacc("TRN2", target_bir_lowering=False, debug=False)
    xh = nc.dram_tensor("xh", [C, HS + 2, Wp], BT, kind="ExternalInput").ap()
    yh = nc.dram_tensor("yh", [C, HS + 2, Wp], BT, kind="ExternalInput").ap()
    qpair = nc.dram_tensor("qpair", [128, 3, C], BT, kind="ExternalInput").ap()
    qsing = nc.dram_tensor("qsing", [C, 3, C], BT, kind="ExternalInput").ap()
    kvpair = nc.dram_tensor("kvpair", [128, 3, 2 * C], BT, kind="ExternalInput").ap()
    kvsing = nc.dram_tensor("kvsing", [C, 3, 2 * C], BT, kind="ExternalInput").ap()
    identb = nc.dram_tensor("identb", [128, 128], BT, kind="ExternalInput").ap()
    if affine:
        gam = nc.dram_tensor("gam", [128, C], BT, kind="ExternalInput").ap()
        bet = nc.dram_tensor("bet", [128, C], BT, kind="ExternalInput").ap()

    gramo = nc.dram_tensor("gramo", [C, NB1, C], FT, kind="ExternalOutput").ap()
    qsso = nc.dram_tensor("qsso", [C, NB1], FT, kind="ExternalOutput").ap()
    ksso = nc.dram_tensor("ksso", [C, NB1], FT, kind="ExternalOutput").ap()
    vout = nc.dram_tensor("vout", [C, HS, W], BT, kind="ExternalOutput").ap()

    with tile.TileContext(nc) as tc:
        import contextlib

        with contextlib.ExitStack() as ctx:
            wp = ctx.enter_context(tc.tile_pool(name="wp", bufs=1))
            io = ctx.enter_context(tc.tile_pool(name="io", bufs=3))
            lnp = ctx.enter_context(tc.tile_pool(name="lnp", bufs=2))
            convp = ctx.enter_context(tc.tile_pool(name="convp", bufs=2))
            dwp = ctx.enter_context(tc.tile_pool(name="dwp", bufs=2))
            lnscr = ctx.enter_context(tc.tile_pool(name="lnscr", bufs=3))
            gramt = ctx.enter_context(tc.tile_pool(name="gramt", bufs=2))
            accp = ctx.enter_context(tc.tile_pool(name="accp", bufs=1))
            ps_c = ctx.enter_context(tc.tile_pool(name="ps_c", bufs=3, space="PSUM"))
            ps_t = ctx.enter_context(tc.tile_pool(name="ps_t", bufs=2, space="PSUM"))
            ps_g = ctx.enter_context(tc.tile_pool(name="ps_g", bufs=1, space="PSUM"))

            qpair_s = wp.tile([128, 3, C], BT)
            nc.sync.dma_start(out=qpair_s, in_=qpair)
            qsing_s = wp.tile([C, 3, C], BT)
            nc.sync.dma_start(out=qsing_s, in_=qsing)
            kvpair_s = wp.tile([128, 3, 2 * C], BT)
            nc.sync.dma_start(out=kvpair_s, in_=kvpair)
            kvsing_s = wp.tile([C, 3, 2 * C], BT)
            nc.sync.dma_start(out=kvsing_s, in_=kvsing)
            id_s = wp.tile([128, 128], BT)
            nc.sync.dma_start(out=id_s, in_=identb)
            eps_s = wp.tile([128, 1], FT)
            nc.vector.memset(eps_s, EPS)
            pools = {"lnscr": lnscr, "ps_t": ps_t, "idb": id_s, "eps": eps_s}
            if affine:
                gam_s = wp.tile([128, C], BT)
                nc.sync.dma_start(out=gam_s, in_=gam)
                bet_s = wp.tile([128, C], BT)
                nc.sync.dma_start(out=bet_s, in_=bet)
                pools["gam_bc"], pools["bet_bc"] = gam_s, bet_s

            gsb = accp.tile([C, NB1, C], FT)
            qss_sb = accp.tile([C, NB1], FT)
            kss_sb = accp.tile([C, NB1], FT)
            scr = accp.tile([2 * C, Hb1, W], BT)

            for band in range(NB1):
                r0 = band * Hb1
                nr = Hb1 + 2
                xb = io.tile([C, nr, Wp], BT, tag="xb")
                nc.sync.dma_start(out=xb, in_=xh[:, r0 : r0 + nr, :])
                yb = io.tile([C, nr, Wp], BT, tag="yb")
                nc.sync.dma_start(out=yb, in_=yh[:, r0 : r0 + nr, :])

                lnx = lnp.tile([128, nr, 260], BT, tag="lnx")
                _zero_pad_cols_s(nc, lnx, nr)
                _ln_into(nc, tc, pools, xb, nr, lnx, affine)
                lny = lnp.tile([128, nr, 260], BT, tag="lny")
                _zero_pad_cols_s(nc, lny, nr)
                _ln_into(nc, tc, pools, yb, nr, lny, affine)

                # fused conv1x1 + depthwise 3x3
                qdw = dwp.tile([C, Hb1, W], BT, tag="qdw")
                kvdw = dwp.tile([2 * C, Hb1, W], BT, tag="kvdw")

                def _ev_q(c, ps):
                    (nc.scalar.copy if c % 2 == 0 else nc.vector.tensor_copy)(
                        qdw[:, 2 * c : 2 * c + 2, :], ps)

                def _ev_kv(c, ps):
                    (nc.scalar.copy if c % 2 == 1 else nc.vector.tensor_copy)(
                        kvdw[:, 2 * c : 2 * c + 2, :], ps)

                _fused_conv(nc, ps_c, qpair_s, qsing_s, lnx, 1, Hb1 // 2, _ev_q, M=C)
                _fused_conv(nc, ps_c, kvpair_s, kvsing_s, lny, 1, Hb1 // 2, _ev_kv, M=2 * C)

                nc.sync.dma_start(out=vout[:, r0 : r0 + Hb1, :], in_=kvdw[C : 2 * C])

                # sum of squares for l2norm
                nc.scalar.activation(
                    scr[0:C], qdw, mybir.ActivationFunctionType.Square,
                    accum_out=qss_sb[:, band : band + 1],
                )
                nc.scalar.activation(
                    scr[0:C], kvdw[0:C], mybir.ActivationFunctionType.Square,
                    accum_out=kss_sb[:, band : band + 1],
                )

                # Gram: transpose q,k chunks then accumulate q^T k
                TQ = Hb1 * 2
                qTs = gramt.tile([128, TQ, C], BT, tag="qTs")
                kTs = gramt.tile([128, TQ, C], BT, tag="kTs")
                for g in range(TQ // 8):
                    ptq = ps_t.tile([128, 8, C], BT, tag="ps_fw")
                    ptk = ps_t.tile([128, 8, C], BT, tag="ps_fw")
                    for j in range(8):
                        t = g * 8 + j
                        row, half = t // 2, t % 2
                        nc.tensor.transpose(ptq[:, j, :], qdw[:, row, 128 * half : 128 * half + 128], id_s[0:64, 0:64])
                        nc.tensor.transpose(ptk[:, j, :], kvdw[0:C, row, 128 * half : 128 * half + 128], id_s[0:64, 0:64])
                    (nc.scalar.copy if g % 2 == 0 else nc.vector.tensor_copy)(qTs[:, g * 8 : g * 8 + 8, :], ptq)
                    (nc.scalar.copy if g % 2 == 1 else nc.vector.tensor_copy)(kTs[:, g * 8 : g * 8 + 8, :], ptk)
                gp = ps_g.tile([C, C], FT, tag="ps_gram")
                for t in range(TQ):
                    nc.tensor.matmul(gp, lhsT=qTs[:, t, :], rhs=kTs[:, t, :], start=(t == 0), stop=(t == TQ - 1))
                nc.scalar.copy(gsb[:, band, :], gp)

            nc.sync.dma_start(out=gramo, in_=gsb)
            nc.sync.dma_start(out=qsso, in_=qss_sb)
            nc.sync.dma_start(out=ksso, in_=kss_sb)

    nc.compile()
    return nc


def _build_k2(affine):
    nc = bacc.Bacc("TRN2", target_bir_lowering=False, debug=False)
    xk = nc.dram_tensor("xk", [C, HS + 4, Wp], FT, kind="ExternalInput").ap()
    vk = nc.dram_tensor("vk", [C, HS + 4, W], BT, kind="ExternalInput").ap()
    ptw = nc.dram_tensor("ptw", [C, C], BT, kind="ExternalInput").ap()
    fpair = nc.dram_tensor("fpair", [128, NG, 3, 128], BT, kind="ExternalInput").ap()
    fsing = nc.dram_tensor("fsing", [C, NG, 3, 128], BT, kind="ExternalInput").ap()
    dw12 = nc.dram_tensor("dw12", [128, NG, 9, 128], BT, kind="ExternalInput").ap()
    wouta = nc.dram_tensor("wouta", [128, C], BT, kind="ExternalInput").ap()
    woutb = nc.dram_tensor("woutb", [42, C], BT, kind="ExternalInput").ap()
    identb = nc.dram_tensor("identb", [128, 128], BT, kind="ExternalInput").ap()
    identf = nc.dram_tensor("identf", [128, 128], FT, kind="ExternalInput").ap()
    if affine:
        gam = nc.dram_tensor("gam", [128, C], BT, kind="ExternalInput").ap()
        bet = nc.dram_tensor("bet", [128, C], BT, kind="ExternalInput").ap()

    out = nc.dram_tensor("out", [C, HS, W], FT, kind="ExternalOutput").ap()

    with tile.TileContext(nc) as tc:
        import contextlib

        with contextlib.ExitStack() as ctx:
            wp = ctx.enter_context(tc.tile_pool(name="wp", bufs=1))
            io = ctx.enter_context(tc.tile_pool(name="io", bufs=2))
            x1p = ctx.enter_context(tc.tile_pool(name="x1p", bufs=2))
            lnp = ctx.enter_context(tc.tile_pool(name="lnp", bufs=2))
            xp = ctx.enter_context(tc.tile_pool(name="xp", bufs=2))
            zp = ctx.enter_context(tc.tile_pool(name="zp", bufs=2))
            gpp = ctx.enter_context(tc.tile_pool(name="gpp", bufs=2))
            thp = ctx.enter_context(tc.tile_pool(name="thp", bufs=2))
            outp = ctx.enter_context(tc.tile_pool(name="outp", bufs=2))
            lnscr = ctx.enter_context(tc.tile_pool(name="lnscr", bufs=2))
            ps_c = ctx.enter_context(tc.tile_pool(name="ps_c", bufs=4, space="PSUM"))
            ps_t = ctx.enter_context(tc.tile_pool(name="ps_t", bufs=2, space="PSUM"))

            pt_s = wp.tile([C, C], BT)
            nc.sync.dma_start(out=pt_s, in_=ptw)
            fpair_s = wp.tile([128, NG, 3, 128], BT)
            nc.sync.dma_start(out=fpair_s, in_=fpair)
            fsing_s = wp.tile([C, NG, 3, 128], BT)
            nc.sync.dma_start(out=fsing_s, in_=fsing)
            dw12_s = wp.tile([128, NG, 9, 128], BT)
            nc.sync.dma_start(out=dw12_s, in_=dw12)
            wouta_s = wp.tile([128, C], BT)
            nc.sync.dma_start(out=wouta_s, in_=wouta)
            woutb_s = wp.tile([42, C], BT)
            nc.sync.dma_start(out=woutb_s, in_=woutb)
            id_s = wp.tile([128, 128], BT)
            nc.sync.dma_start(out=id_s, in_=identb)
            idf_s = wp.tile([128, 128], FT)
            nc.sync.dma_start(out=idf_s, in_=identf)
            eps_s = wp.tile([128, 1], FT)
            nc.vector.memset(eps_s, EPS)
            pools = {"lnscr": lnscr, "ps_t": ps_t, "idb": id_s, "idf": idf_s, "eps": eps_s}
            if affine:
                gam_s = wp.tile([128, C], BT)
                nc.sync.dma_start(out=gam_s, in_=gam)
                bet_s = wp.tile([128, C], BT)
                nc.sync.dma_start(out=bet_s, in_=bet)
                pools["gam_bc"], pools["bet_bc"] = gam_s, bet_s

            for band in range(NB2):
                r0 = band * Hb2
                nr = Hb2 + 4  # x1 rows: interior r0-2 .. r0+Hb2+2
                xb = io.tile([C, nr, Wp], FT, tag="xb")
                nc.sync.dma_start(out=xb, in_=xk[:, r0 : r0 + nr, :])
                vb = io.tile([C, nr, W], BT, tag="vb")
                nc.sync.dma_start(out=vb, in_=vk[:, r0 : r0 + nr, :])

                # x1 = x + P @ v
                x1 = x1p.tile([C, nr, Wp], FT, tag="x1")
                _zero_pad_cols(nc, x1, nr)
                for c in range(nr // 2):
                    pt = ps_c.tile([128, 2, W], FT, tag="ps_mm")
                    nc.tensor.matmul(pt[0:C], lhsT=pt_s, rhs=vb[:, 2 * c : 2 * c + 2, :], start=True, stop=True)
                    nc.vector.tensor_tensor(
                        out=x1[:, 2 * c : 2 * c + 2, 1:257],
                        in0=pt[0:C],
                        in1=xb[:, 2 * c : 2 * c + 2, 1:257],
                        op=AluOpType.add,
                    )

                lnx1 = lnp.tile([128, nr, 260], BT, tag="lnx1")
                _zero_pad_cols_s(nc, lnx1, nr)
                _ln_into(nc, tc, pools, x1, nr, lnx1, affine, src_f32=True)

                # fused w_in conv1x1 + w_dw depthwise -> x1x2 (rows r0-1 .. r0+Hb2+1)
                xts = [xp.tile([128, Hb2 + 2, Wp], BT, tag=f"x12_{g}", name=f"x12_{g}") for g in range(NG)]
                for g in range(NG):
                    _zero_pad_cols(nc, xts[g], Hb2 + 2)
                for g in range(NG):
                    def _ev_x12(c, ps, g=g):
                        (nc.scalar.copy if (c + g) % 2 == 0 else nc.vector.tensor_copy)(
                            xts[g][:, 2 * c : 2 * c + 2, 1:257], ps)
                    _fused_conv(nc, ps_c, fpair_s[:, g, :, :], fsing_s[:, g, :, :],
                                lnx1, 1, (Hb2 + 2) // 2, _ev_x12, M=128)

                # dw1/dw2 depthwise + tanh + residual -> z (rows r0 .. r0+Hb2)
                zts = [zp.tile([128, Hb2, W], BT, tag=f"z{g}", name=f"z{g}") for g in range(NG)]
                z2b = zp.tile([42, Hb2, W], BT, tag="z2b")  # base-0 copy of the G2 x2-tail
                for c in range(Hb2 // 2):
                    for g in range(NG):
                        pt = ps_c.tile([128, 2, W], FT, tag="ps_mm")
                        for t, (dy, dx) in enumerate(TAPS):
                            nc.tensor.matmul(
                                pt,
                                lhsT=dw12_s[:, g, t, :],
                                rhs=xts[g][:, 2 * c + 1 + dy : 2 * c + 3 + dy, 1 + dx : 257 + dx],
                                start=(t == 0),
                                stop=(t == 8),
                            )
                        th = thp.tile([128, 2, W], BT, tag="th")
                        nc.scalar.activation(th, pt, mybir.ActivationFunctionType.Tanh)
                        if g < 2:
                            nc.vector.tensor_tensor(
                                out=zts[g][:, 2 * c : 2 * c + 2, :],
                                in0=th,
                                in1=xts[g][:, 2 * c + 1 : 2 * c + 3, 1:257],
                                op=AluOpType.add,
                            )
                        else:
                            nc.vector.tensor_tensor(
                                out=zts[2][0:42, 2 * c : 2 * c + 2, :],
                                in0=th[0:42],
                                in1=xts[2][0:42, 2 * c + 1 : 2 * c + 3, 1:257],
                                op=AluOpType.add,
                            )
                            nc.vector.tensor_tensor(
                                out=z2b[:, 2 * c : 2 * c + 2, :],
                                in0=th[64:106],
                                in1=xts[2][64:106, 2 * c + 1 : 2 * c + 3, 1:257],
                                op=AluOpType.add,
                            )

                # gating: g = z1 * z2
                g0 = gpp.tile([128, Hb2, W], BT, tag="g0")
                g1 = gpp.tile([42, Hb2, W], BT, tag="g1")
                nc.vector.tensor_tensor(out=g0, in0=zts[0], in1=zts[1], op=AluOpType.mult)
                nc.vector.tensor_tensor(out=g1, in0=zts[2][0:42], in1=z2b, op=AluOpType.mult)

                # w_out + residual
                ot = outp.tile([C, Hb2, W], FT, tag="ot")
                for c in range(Hb2 // 2):
                    pt = ps_c.tile([128, 2, W], FT, tag="ps_mm")
                    nc.tensor.matmul(pt[0:C], lhsT=wouta_s, rhs=g0[:, 2 * c : 2 * c + 2, :], start=True, stop=False)
                    nc.tensor.matmul(pt[0:C], lhsT=woutb_s, rhs=g1[:, 2 * c : 2 * c + 2, :], start=False, stop=True)
                    nc.vector.tensor_tensor(
                        out=ot[:, 2 * c : 2 * c + 2, :],
                        in0=pt[0:C],
                        in1=x1[:, 2 * c + 2 : 2 * c + 4, 1:257],
                        op=AluOpType.add,
                    )
                nc.sync.dma_start(out=out[:, r0 : r0 + Hb2, :], in_=ot)

    nc.compile()
    return nc


# ---------------------------------------------------------------- host logic

_CACHE = {}


def _programs(affine):
    key = ("k", affine)
    if key not in _CACHE:
        _CACHE[key] = (_build_k1(affine), _build_k2(affine))
    return _CACHE[key]


def _diag_blocks(w, place):
    """w: [340] per-tap vector -> [3,128,128] diag matrices per placed group."""
    out = np.zeros((NG, 128, 128), F32)
    for s, ch in enumerate(place):
        if ch >= 0:
            out[s // 128, s % 128, s % 128] = w[ch]
    return out


def kernel(x, y, ln_w, ln_b, temperature, wq, wq_dw, wkv, wkv_dw, w_proj,
           w_in, w_dw, w_dw1, w_dw2, w_out):
    x = np.asarray(x, F32)
    y = np.asarray(y, F32)
    ln_w = np.asarray(ln_w, F32)
    ln_b = np.asarray(ln_b, F32)
    temperature = np.asarray(temperature, F32)
    wq = np.asarray(wq, F32)
    wq_dw = np.asarray(wq_dw, F32)
    wkv = np.asarray(wkv, F32)
    wkv_dw = np.asarray(wkv_dw, F32)
    w_proj = np.asarray(w_proj, F32)
    w_in = np.asarray(w_in, F32)
    w_dw = np.asarray(w_dw, F32)
    w_dw1 = np.asarray(w_dw1, F32)
    w_dw2 = np.asarray(w_dw2, F32)
    w_out = np.asarray(w_out, F32)

    affine = not (np.allclose(ln_w, 1.0) and np.allclose(ln_b, 0.0))
    k1, k2 = _programs(affine)

    # ---------- launch 1: q/k gram + norms + v
    xpad = np.zeros((B, C, H + 4, Wp), F32)
    xpad[:, :, 2 : 2 + H, 1 : 1 + W] = x
    ypad = np.zeros((B, C, H + 4, Wp), F32)
    ypad[:, :, 2 : 2 + H, 1 : 1 + W] = y

    def _fuse_pairs(w1x1, wdw):
        # w1x1: [O, C]; wdw: [O, 1, 3, 3] -> pairs [128, 3, O], singles [C, 3, O]
        O = w1x1.shape[0]
        pairs = np.zeros((128, 3, O), F32)
        sings = np.zeros((C, 3, O), F32)
        for p in range(3):
            pairs[0:C, p, :] = (w1x1 * wdw[:, 0, p, 0][:, None]).T
            pairs[C:128, p, :] = (w1x1 * wdw[:, 0, p, 2][:, None]).T
            sings[:, p, :] = (w1x1 * wdw[:, 0, p, 1][:, None]).T
        return pairs.astype(BF16), sings.astype(BF16)

    qpair, qsing = _fuse_pairs(wq, wq_dw)
    kvpair, kvsing = _fuse_pairs(wkv, wkv_dw)
    common1 = {
        "qpair": qpair,
        "qsing": qsing,
        "kvpair": kvpair,
        "kvsing": kvsing,
        "identb": np.eye(128).astype(BF16),
    }
    if affine:
        common1["gam"] = np.broadcast_to(ln_w[None, :], (128, C)).astype(BF16).copy()
        common1["bet"] = np.broadcast_to(ln_b[None, :], (128, C)).astype(BF16).copy()

    in_maps1 = []
    for core in range(NCORES):
        b, h = core // 2, core % 2
        rs = 2 + h * HS - 1  # padded-coords start row for halo-1 slab
        m = dict(common1)
        m["xh"] = np.ascontiguousarray(xpad[b, :, rs : rs + HS + 2, :]).astype(BF16)
        m["yh"] = np.ascontiguousarray(ypad[b, :, rs : rs + HS + 2, :]).astype(BF16)
        in_maps1.append(m)

    res1 = bass_utils.run_bass_kernel_spmd(k1, in_maps1, core_ids=list(range(NCORES)))

    # ---------- host combine: attention softmax -> P = w_proj @ blockdiag(A)
    pts = []
    vfull = np.zeros((B, C, H, W), BF16)
    for b in range(B):
        r0, r1 = res1.results[2 * b], res1.results[2 * b + 1]
        G = r0["gramo"].astype(np.float64).sum(1) + r1["gramo"].astype(np.float64).sum(1)
        qss = r0["qsso"].astype(np.float64).sum(1) + r1["qsso"].astype(np.float64).sum(1)
        kss = r0["ksso"].astype(np.float64).sum(1) + r1["ksso"].astype(np.float64).sum(1)
        nq = np.maximum(np.sqrt(qss), 1e-12)
        nk = np.maximum(np.sqrt(kss), 1e-12)
        A = np.zeros((C, C), np.float64)
        for hh in range(HEADS):
            sl = slice(hh * CH, (hh + 1) * CH)
            logits = temperature[hh, 0, 0] * (G[sl, sl] / np.outer(nq[sl], nk[sl]))
            e = np.exp(logits - logits.max(axis=-1, keepdims=True))
            A[sl, sl] = e / e.sum(axis=-1, keepdims=True)
        P = w_proj.astype(np.float64) @ A
        pts.append(np.ascontiguousarray(P.T).astype(BF16))
        vfull[b, :, 0:HS] = r0["vout"]
        vfull[b, :, HS:H] = r1["vout"]

    # ---------- launch 2: x1 = x + P v ; IEL
    vpad = np.zeros((B, C, H + 4, W), BF16)
    vpad[:, :, 2 : 2 + H, :] = vfull

    w_in_p = np.zeros((NG * 128, C), F32)
    w_dw_p = np.zeros((NG * 128, 3, 3), F32)
    w12 = np.concatenate([w_dw1[:, 0], w_dw2[:, 0]], axis=0)  # [340,3,3]
    w12_p = np.zeros((NG * 128, 3, 3), F32)
    for s, ch in enumerate(PLACE340):
        if ch >= 0:
            w_in_p[s] = w_in[ch]
            w_dw_p[s] = w_dw[ch, 0]
            w12_p[s] = w12[ch]
    fpair = np.zeros((128, NG, 3, 128), F32)
    fsing = np.zeros((C, NG, 3, 128), F32)
    for g in range(NG):
        sl = slice(g * 128, (g + 1) * 128)
        for p in range(3):
            fpair[0:C, g, p, :] = (w_in_p[sl] * w_dw_p[sl, p, 0][:, None]).T
            fpair[C:128, g, p, :] = (w_in_p[sl] * w_dw_p[sl, p, 2][:, None]).T
            fsing[:, g, p, :] = (w_in_p[sl] * w_dw_p[sl, p, 1][:, None]).T
    dw12_d = np.zeros((128, NG, 9, 128), F32)
    for t in range(9):
        ty, tx = t // 3, t % 3
        d1 = _diag_blocks(w12[:, ty, tx], PLACE340)
        for g in range(NG):
            dw12_d[:, g, t, :] = d1[g]

    common2 = {
        "fpair": fpair.astype(BF16),
        "fsing": fsing.astype(BF16),
        "dw12": dw12_d.astype(BF16),
        "wouta": np.ascontiguousarray(w_out.T[0:128]).astype(BF16),
        "woutb": np.ascontiguousarray(w_out.T[128:170]).astype(BF16),
        "identb": np.eye(128).astype(BF16),
        "identf": np.eye(128).astype(F32),
    }
    if affine:
        common2["gam"] = common1["gam"]
        common2["bet"] = common1["bet"]

    in_maps2 = []
    for core in range(NCORES):
        b, h = core // 2, core % 2
        rs = 2 + h * HS - 2
        m = dict(common2)
        m["xk"] = np.ascontiguousarray(xpad[b, :, rs : rs + HS + 4, :])
        m["vk"] = np.ascontiguousarray(vpad[b, :, rs : rs + HS + 4, :])
        m["ptw"] = pts[b]
        in_maps2.append(m)

    res2 = bass_utils.run_bass_kernel_spmd(k2, in_maps2, core_ids=list(range(NCORES)))

    out = np.zeros((B, C, H, W), F32)
    for core in range(NCORES):
        b, h = core // 2, core % 2
        out[b, :, h * HS : (h + 1) * HS, :] = res2.results[core]["out"]
    return out



# revision 7
# speedup vs baseline: 1.5133x; 1.5133x over previous
"""Trainium2 Bass kernel for nn_CSDC_8246337208509 (I_LCA block: CAB cross-attention + IEL gated FFN).

Contract: kernel(**inputs) takes FULL unsharded inputs, returns FULL output.
Sharding: 8 cores = 4 batches x 2 spatial halves (128 rows of H each).
Two device launches with a tiny host-side combine (attention softmax over 8x8
per-head Gram matrices) between them.
"""

import contextlib
import sys

import numpy as np

try:
    import concourse.bass as bass  # noqa: F401
except Exception:  # pragma: no cover
    sys.path.insert(0, "/opt/trn_rl_repo")
    sys.path.insert(0, "/root/.axon_site/_ro/trn_rl_repo")

import concourse.bacc as bacc
import concourse.tile as tile
from concourse import mybir
from concourse import bass_utils
from concourse.alu_op_type import AluOpType
import ml_dtypes

BF16 = ml_dtypes.bfloat16
F32 = np.float32
BT = mybir.dt.bfloat16
FT = mybir.dt.float32

B, C, H, W = 4, 64, 256, 256
HEADS, CH = 8, 8
HID = 170
EPS = 1e-6
Wp = W + 2  # 258, zero col at 0 and 257
HS = H // 2  # 128 interior rows per core
Hb1 = 16  # k1 band interior rows
NB1 = HS // Hb1
Hb2 = 8  # k2 band interior rows
NB2 = HS // Hb2
NCORES = 8

TAPS = [(ty - 1, tx - 1) for ty in range(3) for tx in range(3)]  # (dy, dx), t = ty*3+tx

F8 = mybir.dt.float8e4
E4M3 = ml_dtypes.float8_e4m3
Wf = 272  # fp8 padded row stride (mult of 16)
SC1E = 10  # fused-d weight scale exponent
SDE = 5    # fp8 d storage scale exponent
SC2E = 9   # dw12 weight scale exponent
PH = 68    # k2 pass tile height
NCL = 34
NCD = 33
NCO = 32
BAND = 16
NBAND = 3
DRMODE = mybir.MatmulPerfMode.DoubleRow


# channel placement for the 340-wide IEL stream into 3 groups of 128 partitions:
# G0 = x1[0:128]; G1 = x2[0:128]; G2: slots 0..41 = x1[128:170], slots 64..105 =
# x2[128:170] (partition bases must be 32-aligned, so the x2 tail sits at 64).
PLACE340 = [-1] * 384
for _i in range(128):
    PLACE340[_i] = _i           # G0
    PLACE340[128 + _i] = 170 + _i  # G1
for _i in range(42):
    PLACE340[256 + _i] = 128 + _i  # G2 low: x1 tail
    PLACE340[256 + 64 + _i] = 298 + _i  # G2 high: x2 tail
NG = 3


# ---------------------------------------------------------------- device code

def _ln_into(nc, tc, pools, src, nrows, dst, affine, src_f32=False):
    """Channels-first LayerNorm of src[:, :nrows, 1:257] -> dst (S-stacked bf16).

    src: [64, nrows, 258] bf16 tile. dst: [128, nrows, 260] S-layout tile whose
    pad cols are already zeroed: top half dst[0:64, r, c] = ln[r, c-1] (written
    at cols 2:258), bottom half dst[64:128, r, c] = ln[r, c+1] (gpsimd copy).
    All transposes ride the DMA xbar (bf16), not the PE.
    """
    lnscr = pools["lnscr"]
    T = nrows * 2
    xTs = lnscr.tile([128, T, 64], BT, tag="ln_xTs")
    xnT = lnscr.tile([128, T, 64], BT, tag="ln_xnT")
    st = lnscr.tile([128, T, 6], FT, tag="ln_st")
    mv = lnscr.tile([128, T, 2], FT, tag="ln_mv")
    sr = lnscr.tile([128, T, 1], FT, tag="ln_sr")
    ri = lnscr.tile([128, T, 1], FT, tag="ln_ri")

    ps_t = pools["ps_t"]
    ident = pools["idf"] if src_f32 else pools["idb"]
    for g in range((T + 7) // 8):
        n = min(8, T - g * 8)
        pt = ps_t.tile([128, 8, 64], FT if src_f32 else BT, tag="ps_fw")
        for j in range(n):
            t = g * 8 + j
            row, half = t // 2, t % 2
            nc.tensor.transpose(
                pt[:, j, :],
                src[:, row, 1 + 128 * half : 1 + 128 * half + 128],
                ident[0:64, 0:64],
            )
        (nc.scalar.copy if g % 2 == 0 else nc.vector.tensor_copy)(
            xTs[:, g * 8 : g * 8 + n, :], pt[:, 0:n, :]
        )
    for t in range(T):
        nc.vector.bn_stats(st[:, t, :], xTs[:, t, :])
        nc.vector.bn_aggr(mv[:, t, :], st[:, t, :])
    nc.scalar.activation(sr, mv[:, :, 1:2], mybir.ActivationFunctionType.Sqrt, bias=pools["eps"])
    nc.vector.reciprocal(ri, sr)
    for t in range(T):
        nc.vector.tensor_scalar(
            out=xnT[:, t, :],
            in0=xTs[:, t, :],
            scalar1=mv[:, t, 0:1],
            scalar2=ri[:, t, 0:1],
            op0=AluOpType.subtract,
            op1=AluOpType.mult,
        )
    if affine:
        gam_bc, bet_bc = pools["gam_bc"], pools["bet_bc"]
        for t in range(T):
            nc.vector.tensor_tensor(out=xnT[:, t, :], in0=xnT[:, t, :], in1=gam_bc, op=AluOpType.mult)
            nc.vector.tensor_tensor(out=xnT[:, t, :], in0=xnT[:, t, :], in1=bet_bc, op=AluOpType.add)
    # transpose back (PE) into the S-layout top half, then gpsimd-fill the bottom
    for g in range((T + 3) // 4):
        pb = ps_t.tile([128, 2, 256], BT, tag="ps_bw")
        for j in range(4):
            t = g * 4 + j
            nc.tensor.transpose(
                pb[0:64, j // 2, 128 * (j % 2) : 128 * (j % 2) + 128],
                xnT[:, t, :],
                pools["idb"],
            )
        (nc.scalar.copy if g % 2 == 0 else nc.vector.tensor_copy)(
            dst[0:64, g * 2 : g * 2 + 2, 2:258], pb[0:64]
        )
        nc.gpsimd.tensor_copy(
            dst[64:128, g * 2 : g * 2 + 2, 0:256], dst[0:64, g * 2 : g * 2 + 2, 2:258]
        )


def _zero_pad_cols(nc, t, nrows):
    nc.gpsimd.memset(t[:, 0:nrows, 0:1], 0.0)
    nc.gpsimd.memset(t[:, 0:nrows, 257:258], 0.0)


def _zero_pad_cols_s(nc, t, nrows):
    # S-stacked layout [128, nrows, 260]: top half holds u[c-1], bottom u[c+1]
    nc.gpsimd.memset(t[0:64, 0:nrows, 0:2], 0.0)
    nc.gpsimd.memset(t[0:64, 0:nrows, 258:260], 0.0)
    nc.gpsimd.memset(t[64:128, 0:nrows, 256:260], 0.0)


# fused conv1x1+dw3x3: 3 K=128 pair-matmuls + 3 K=64 single-matmuls per chunk.
# S: [128, nr, 260] stacked input; out rows j correspond to S rows j+roff.
def _fused_conv(nc, ps_pool, pairs, sings, S, roff, nchunks, evict, M=128):
    for c in range(nchunks):
        pt = ps_pool.tile([128, 2, W], FT, tag="ps_mm")
        for p in range(3):
            dy = p - 1
            nc.tensor.matmul(
                pt[0:M],
                lhsT=pairs[:, p, :],
                rhs=S[:, roff + 2 * c + dy : roff + 2 * c + dy + 2, 1:257],
                start=(p == 0),
                stop=False,
            )
        for i in range(3):
            dy = i - 1
            nc.tensor.matmul(
                pt[0:M],
                lhsT=sings[:, i, :],
                rhs=S[0:64, roff + 2 * c + dy : roff + 2 * c + dy + 2, 2:258],
                start=False,
                stop=(i == 2),
            )
        evict(c, pt[0:M])


def _build_k1(affine):
    nc = bacc.Bacc("TRN2", target_bir_lowering=False, debug=False)
    xh = nc.dram_tensor("xh", [C, HS + 2, Wp], BT, kind="ExternalInput").ap()
    yh = nc.dram_tensor("yh", [C, HS + 2, Wp], BT, kind="ExternalInput").ap()
    qpair = nc.dram_tensor("qpair", [128, 3, C], BT, kind="ExternalInput").ap()
    qsing = nc.dram_tensor("qsing", [C, 3, C], BT, kind="ExternalInput").ap()
    kvpair = nc.dram_tensor("kvpair", [128, 3, 2 * C], BT, kind="ExternalInput").ap()
    kvsing = nc.dram_tensor("kvsing", [C, 3, 2 * C], BT, kind="ExternalInput").ap()
    identb = nc.dram_tensor("identb", [128, 128], BT, kind="ExternalInput").ap()
    if affine:
        gam = nc.dram_tensor("gam", [128, C], BT, kind="ExternalInput").ap()
        bet = nc.dram_tensor("bet", [128, C], BT, kind="ExternalInput").ap()

    gramo = nc.dram_tensor("gramo", [C, NB1, C], FT, kind="ExternalOutput").ap()
    qsso = nc.dram_tensor("qsso", [C, NB1], FT, kind="ExternalOutput").ap()
    ksso = nc.dram_tensor("ksso", [C, NB1], FT, kind="ExternalOutput").ap()
    vout = nc.dram_tensor("vout", [C, HS, W], BT, kind="ExternalOutput").ap()

    with tile.TileContext(nc) as tc:
        import contextlib

        with contextlib.ExitStack() as ctx:
            wp = ctx.enter_context(tc.tile_pool(name="wp", bufs=1))
            io = ctx.enter_context(tc.tile_pool(name="io", bufs=3))
            lnp = ctx.enter_context(tc.tile_pool(name="lnp", bufs=2))
            convp = ctx.enter_context(tc.tile_pool(name="convp", bufs=2))
            dwp = ctx.enter_context(tc.tile_pool(name="dwp", bufs=2))
            lnscr = ctx.enter_context(tc.tile_pool(name="lnscr", bufs=3))
            gramt = ctx.enter_context(tc.tile_pool(name="gramt", bufs=2))
            accp = ctx.enter_context(tc.tile_pool(name="accp", bufs=1))
            ps_c = ctx.enter_context(tc.tile_pool(name="ps_c", bufs=3, space="PSUM"))
            ps_t = ctx.enter_context(tc.tile_pool(name="ps_t", bufs=2, space="PSUM"))
            ps_g = ctx.enter_context(tc.tile_pool(name="ps_g", bufs=1, space="PSUM"))

            qpair_s = wp.tile([128, 3, C], BT)
            nc.sync.dma_start(out=qpair_s, in_=qpair)
            qsing_s = wp.tile([C, 3, C], BT)
            nc.sync.dma_start(out=qsing_s, in_=qsing)
            kvpair_s = wp.tile([128, 3, 2 * C], BT)
            nc.sync.dma_start(out=kvpair_s, in_=kvpair)
            kvsing_s = wp.tile([C, 3, 2 * C], BT)
            nc.sync.dma_start(out=kvsing_s, in_=kvsing)
            id_s = wp.tile([128, 128], BT)
            nc.sync.dma_start(out=id_s, in_=identb)
            eps_s = wp.tile([128, 1], FT)
            nc.vector.memset(eps_s, EPS)
            pools = {"lnscr": lnscr, "ps_t": ps_t, "idb": id_s, "eps": eps_s}
            if affine:
                gam_s = wp.tile([128, C], BT)
                nc.sync.dma_start(out=gam_s, in_=gam)
                bet_s = wp.tile([128, C], BT)
                nc.sync.dma_start(out=bet_s, in_=bet)
                pools["gam_bc"], pools["bet_bc"] = gam_s, bet_s

            gsb = accp.tile([C, NB1, C], FT)
            qss_sb = accp.tile([C, NB1], FT)
            kss_sb = accp.tile([C, NB1], FT)
            scr = accp.tile([2 * C, Hb1, W], BT)

            for band in range(NB1):
                r0 = band * Hb1
                nr = Hb1 + 2
                xb = io.tile([C, nr, Wp], BT, tag="xb")
                nc.sync.dma_start(out=xb, in_=xh[:, r0 : r0 + nr, :])
                yb = io.tile([C, nr, Wp], BT, tag="yb")
                nc.sync.dma_start(out=yb, in_=yh[:, r0 : r0 + nr, :])

                lnx = lnp.tile([128, nr, 260], BT, tag="lnx")
                _zero_pad_cols_s(nc, lnx, nr)
                _ln_into(nc, tc, pools, xb, nr, lnx, affine)
                lny = lnp.tile([128, nr, 260], BT, tag="lny")
                _zero_pad_cols_s(nc, lny, nr)
                _ln_into(nc, tc, pools, yb, nr, lny, affine)

                # fused conv1x1 + depthwise 3x3
                qdw = dwp.tile([C, Hb1, W], BT, tag="qdw")
                kvdw = dwp.tile([2 * C, Hb1, W], BT, tag="kvdw")

                def _ev_q(c, ps):
                    (nc.scalar.copy if c % 2 == 0 else nc.vector.tensor_copy)(
                        qdw[:, 2 * c : 2 * c + 2, :], ps)

                def _ev_kv(c, ps):
                    (nc.scalar.copy if c % 2 == 1 else nc.vector.tensor_copy)(
                        kvdw[:, 2 * c : 2 * c + 2, :], ps)

                _fused_conv(nc, ps_c, qpair_s, qsing_s, lnx, 1, Hb1 // 2, _ev_q, M=C)
                _fused_conv(nc, ps_c, kvpair_s, kvsing_s, lny, 1, Hb1 // 2, _ev_kv, M=2 * C)

                nc.sync.dma_start(out=vout[:, r0 : r0 + Hb1, :], in_=kvdw[C : 2 * C])

                # sum of squares for l2norm
                nc.scalar.activation(
                    scr[0:C], qdw, mybir.ActivationFunctionType.Square,
                    accum_out=qss_sb[:, band : band + 1],
                )
                nc.scalar.activation(
                    scr[0:C], kvdw[0:C], mybir.ActivationFunctionType.Square,
                    accum_out=kss_sb[:, band : band + 1],
                )

                # Gram: transpose q,k chunks then accumulate q^T k
                TQ = Hb1 * 2
                qTs = gramt.tile([128, TQ, C], BT, tag="qTs")
                kTs = gramt.tile([128, TQ, C], BT, tag="kTs")
                for g in range(TQ // 8):
                    ptq = ps_t.tile([128, 8, C], BT, tag="ps_fw")
                    ptk = ps_t.tile([128, 8, C], BT, tag="ps_fw")
                    for j in range(8):
                        t = g * 8 + j
                        row, half = t // 2, t % 2
                        nc.tensor.transpose(ptq[:, j, :], qdw[:, row, 128 * half : 128 * half + 128], id_s[0:64, 0:64])
                        nc.tensor.transpose(ptk[:, j, :], kvdw[0:C, row, 128 * half : 128 * half + 128], id_s[0:64, 0:64])
                    (nc.scalar.copy if g % 2 == 0 else nc.vector.tensor_copy)(qTs[:, g * 8 : g * 8 + 8, :], ptq)
                    (nc.scalar.copy if g % 2 == 1 else nc.vector.tensor_copy)(kTs[:, g * 8 : g * 8 + 8, :], ptk)
                gp = ps_g.tile([C, C], FT, tag="ps_gram")
                for t in range(TQ):
                    nc.tensor.matmul(gp, lhsT=qTs[:, t, :], rhs=kTs[:, t, :], start=(t == 0), stop=(t == TQ - 1))
                nc.scalar.copy(gsb[:, band, :], gp)

            nc.sync.dma_start(out=gramo, in_=gsb)
            nc.sync.dma_start(out=qsso, in_=qss_sb)
            nc.sync.dma_start(out=ksso, in_=kss_sb)

    nc.compile()
    return nc


def _build_k2(affine):
    nc = bacc.Bacc("TRN2", target_bir_lowering=False, debug=False)
    xk = nc.dram_tensor("xk", [C, 132, Wp], BT, kind="ExternalInput").ap()
    vk = nc.dram_tensor("vk", [C, 132, W], BT, kind="ExternalInput").ap()
    ptw = nc.dram_tensor("ptw", [C, C], BT, kind="ExternalInput").ap()
    vsel = nc.dram_tensor("vsel", [C, BAND, C], BT, kind="ExternalInput").ap()
    bsel = nc.dram_tensor("bsel", [BAND, BAND, C], BT, kind="ExternalInput").ap()
    fdrp = nc.dram_tensor("fdrp", [128, NG, 2, 128], F8, kind="ExternalInput").ap()
    fdrs = nc.dram_tensor("fdrs", [C, NG, 2, 128], F8, kind="ExternalInput").ap()
    fnp = nc.dram_tensor("fnp", [128, NG, 128], F8, kind="ExternalInput").ap()
    fns = nc.dram_tensor("fns", [C, NG, 128], F8, kind="ExternalInput").ap()
    wdr = nc.dram_tensor("wdr", [128, NG, 3, 2, 128], F8, kind="ExternalInput").ap()
    wn = nc.dram_tensor("wn", [128, NG, 3, 128], F8, kind="ExternalInput").ap()
    wouta = nc.dram_tensor("wouta", [128, C], BT, kind="ExternalInput").ap()
    woutb = nc.dram_tensor("woutb", [42, C], BT, kind="ExternalInput").ap()
    if affine:
        gamv = nc.dram_tensor("gamv", [C, 1], FT, kind="ExternalInput").ap()
        betv = nc.dram_tensor("betv", [C, 1], FT, kind="ExternalInput").ap()

    out = nc.dram_tensor("out", [C, HS, W], BT, kind="ExternalOutput").ap()

    with tile.TileContext(nc) as tc:
        with contextlib.ExitStack() as ctx:
            wp = ctx.enter_context(tc.tile_pool(name="wp", bufs=1))
            big = ctx.enter_context(tc.tile_pool(name="big", bufs=1))
            ring = ctx.enter_context(tc.tile_pool(name="ring", bufs=3))
            srng = ctx.enter_context(tc.tile_pool(name="srng", bufs=2))
            ps_pu = ctx.enter_context(tc.tile_pool(name="ps_pu", bufs=2, space="PSUM"))
            ps_ro = ctx.enter_context(tc.tile_pool(name="ps_ro", bufs=2, space="PSUM"))
            ps_var = ctx.enter_context(tc.tile_pool(name="ps_var", bufs=1, space="PSUM"))
            ps_d = ctx.enter_context(tc.tile_pool(name="ps_d", bufs=2, space="PSUM"))
            ps_t = ctx.enter_context(tc.tile_pool(name="ps_t", bufs=1, space="PSUM"))

            # ------------------------------------------------ persistent weights
            ptw_s = wp.tile([C, C], BT)
            nc.sync.dma_start(out=ptw_s, in_=ptw)
            vsel_s = wp.tile([C, BAND, C], BT)
            nc.sync.dma_start(out=vsel_s, in_=vsel)
            bsel_s = wp.tile([BAND, BAND, C], BT)
            nc.sync.dma_start(out=bsel_s, in_=bsel)
            fdrp_s = wp.tile([128, NG, 2, 128], F8)
            nc.sync.dma_start(out=fdrp_s, in_=fdrp)
            fdrs_s = wp.tile([C, NG, 2, 128], F8)
            nc.sync.dma_start(out=fdrs_s, in_=fdrs)
            fnp_s = wp.tile([128, NG, 128], F8)
            nc.sync.dma_start(out=fnp_s, in_=fnp)
            fns_s = wp.tile([C, NG, 128], F8)
            nc.sync.dma_start(out=fns_s, in_=fns)
            wdr_s = wp.tile([128, NG, 3, 2, 128], F8)
            nc.sync.dma_start(out=wdr_s, in_=wdr)
            wn_s = wp.tile([128, NG, 3, 128], F8)
            nc.sync.dma_start(out=wn_s, in_=wn)
            wouta_s = wp.tile([128, C], BT)
            nc.sync.dma_start(out=wouta_s, in_=wouta)
            woutb_s = wp.tile([42, C], BT)
            nc.sync.dma_start(out=woutb_s, in_=woutb)
            usel_s = wp.tile([C, C], BT)
            nc.vector.memset(usel_s, 1.0 / 64.0)
            ones1 = wp.tile([1, C], BT)
            nc.vector.memset(ones1, 1.0)
            eps_s = wp.tile([128, 1], FT)
            nc.vector.memset(eps_s, EPS)
            if affine:
                gam_s = wp.tile([C, 1], FT)
                nc.sync.dma_start(out=gam_s, in_=gamv)
                bet_s = wp.tile([C, 1], FT)
                nc.sync.dma_start(out=bet_s, in_=betv)

            for p in range(2):
                d0 = 64 * p  # dram row of local row 0

                xt = big.tile([128, PH, Wp], BT, tag="x1", name=f"x1_{p}")
                x1t = xt[0:C]
                xcs = xt[C:128, :, 1:257]
                lnS = big.tile([128, PH, Wf], F8, tag="lnS", name=f"lnS_{p}")
                dts = [
                    big.tile([128, PH, Wf], F8, tag=f"d{g}", name=f"d{g}_{p}")
                    for g in range(NG)
                ]
                rband = big.tile([BAND, NBAND, 512], BT, tag="rband", name=f"rband_{p}")

                nc.gpsimd.memset(lnS[0:C, :, 0:2], 0.0)
                nc.gpsimd.memset(lnS[0:C, :, 258:Wf], 0.0)
                nc.gpsimd.memset(lnS[C:128, :, 256:Wf], 0.0)
                for g in range(NG):
                    nc.gpsimd.memset(dts[g][:, :, 0:1], 0.0)
                    nc.gpsimd.memset(dts[g][:, :, 257:Wf], 0.0)

                nrow_grp = [8] * 8 + [4]
                xbs = vbs = None
                vps_cur = None
                ot_cur = None

                for i in range(NCO + 20):
                    # ---------------- stage 1
                    c = i
                    if c < NCL:
                        if c % 4 == 0:
                            g4 = c // 4
                            nr = nrow_grp[g4]
                            xb = ring.tile([C, 8, Wp], BT, tag="xb", name="xb")
                            nc.sync.dma_start(
                                out=xb[:, 0:nr, :],
                                in_=xk[:, d0 + 8 * g4 : d0 + 8 * g4 + nr, :],
                            )
                            vb = ring.tile([C, 8, W], BT, tag="vb", name="vb")
                            nc.sync.dma_start(
                                out=vb[:, 0:nr, :],
                                in_=vk[:, d0 + 8 * g4 : d0 + 8 * g4 + nr, :],
                            )
                            xbs, vbs = xb, vb
                        lr = c % 4 * 2
                        pu = ps_pu.tile([128, 2, W], FT, tag="pu", name="pu")
                        nc.tensor.matmul(
                            pu[0:C], lhsT=ptw_s, rhs=vbs[:, lr : lr + 2, :],
                            start=True, stop=True,
                        )
                        nc.vector.tensor_tensor(
                            out=x1t[:, 2 * c : 2 * c + 2, 1:257],
                            in0=pu[0:C],
                            in1=xbs[:, lr : lr + 2, 1:257],
                            op=AluOpType.add,
                        )
                        nc.tensor.matmul(
                            pu[C:128], lhsT=usel_s,
                            rhs=x1t[:, 2 * c : 2 * c + 2, 1:257],
                            start=True, stop=True,
                        )
                        nc.vector.tensor_tensor(
                            out=xcs[:, 2 * c : 2 * c + 2, :],
                            in0=x1t[:, 2 * c : 2 * c + 2, 1:257],
                            in1=pu[C:128],
                            op=AluOpType.subtract,
                        )
                        xq = srng.tile([C, 2, W], BT, tag="xq", name="xq")
                        nc.vector.tensor_tensor(
                            out=xq, in0=xcs[:, 2 * c : 2 * c + 2, :],
                            in1=xcs[:, 2 * c : 2 * c + 2, :], op=AluOpType.mult,
                        )
                        B = c // BAND
                        j = c % BAND
                        if j == 0:
                            vps_cur = ps_var.tile([C, 512], FT, tag="vps", name="vps")
                        nc.tensor.matmul(
                            vps_cur, lhsT=vsel_s[:, j, :], rhs=xq,
                            start=(j == 0), stop=(j == BAND - 1 or c == NCL - 1),
                        )
                        if j == BAND - 1 or c == NCL - 1:
                            sb = srng.tile([BAND, 512], BT, tag="sb", name="sb")
                            nc.scalar.activation(
                                sb, vps_cur[0:BAND], mybir.ActivationFunctionType.Sqrt,
                                bias=eps_s[0:BAND],
                            )
                            with nc.allow_low_precision(reason="bf16 rsqrt rows"):
                                nc.vector.reciprocal(rband[:, B, :], sb)

                    # ---------------- stage 2: apply (lag 16)
                    jc = i - BAND
                    if 0 <= jc < NCL:
                        B = jc // BAND
                        jj = jc % BAND
                        ro = ps_ro.tile([128, 2, W], FT, tag="ro", name="ro")
                        nc.tensor.matmul(
                            ro[0:C], lhsT=bsel_s[:, jj, :],
                            rhs=rband[:, B, :],
                            start=True, stop=True,
                        )
                        if affine:
                            tmp = srng.tile([C, 2, W], BT, tag="tmp", name="tmp")
                            nc.vector.tensor_tensor(
                                out=tmp, in0=xcs[:, 2 * jc : 2 * jc + 2, :],
                                in1=ro[0:C], op=AluOpType.mult,
                            )
                            nc.vector.tensor_scalar(
                                out=lnS[0:C, 2 * jc : 2 * jc + 2, 2:258],
                                in0=tmp, scalar1=gam_s, scalar2=bet_s,
                                op0=AluOpType.mult, op1=AluOpType.add,
                            )
                        else:
                            nc.vector.tensor_tensor(
                                out=lnS[0:C, 2 * jc : 2 * jc + 2, 2:258],
                                in0=xcs[:, 2 * jc : 2 * jc + 2, :],
                                in1=ro[0:C], op=AluOpType.mult,
                            )
                        nc.gpsimd.tensor_copy(
                            lnS[C:128, 2 * jc : 2 * jc + 2, 0:256],
                            lnS[0:C, 2 * jc : 2 * jc + 2, 2:258],
                        )
                        ro_apply = ro  # keep handle: wout reuses other half
                    # ---------------- stage 3: fused w_in + dw3 -> d (lag 18)
                    k = i - BAND - 2
                    if 0 <= k < NCD:
                        prhs = lnS[:, 2 * k : 2 * k + 4, 1:257].rearrange(
                            "p (a b) w -> p a b w", a=2
                        )
                        srhs = lnS[0:C, 2 * k : 2 * k + 4, 2:258].rearrange(
                            "p (a b) w -> p a b w", a=2
                        )
                        for g in range(NG):
                            dp = ps_d.tile([128, 2, W], FT, tag="dp", name="dp")
                            nc.tensor.matmul(
                                dp, lhsT=fdrp_s[:, g], rhs=prhs,
                                start=True, stop=False, perf_mode=DRMODE,
                            )
                            nc.tensor.matmul(
                                dp, lhsT=fdrs_s[:, g], rhs=srhs,
                                start=False, stop=False, perf_mode=DRMODE,
                            )
                            nc.tensor.matmul(
                                dp, lhsT=fnp_s[:, g],
                                rhs=lnS[:, 2 * k + 1 : 2 * k + 3, 1:257],
                                start=False, stop=False,
                            )
                            nc.tensor.matmul(
                                dp, lhsT=fns_s[:, g],
                                rhs=lnS[0:C, 2 * k + 1 : 2 * k + 3, 2:258],
                                start=False, stop=True,
                            )
                            nc.scalar.activation(
                                dts[g][:, 2 * k + 1 : 2 * k + 3, 1:257],
                                dp, mybir.ActivationFunctionType.Copy,
                                scale=float(2.0 ** (SDE - SC1E)),
                            )

                    # ---------------- stage 4 (lag 20)
                    m = i - BAND - 4
                    if 0 <= m < NCO:
                        zts = []
                        for g in range(NG):
                            tp = ps_t.tile([128, 2, W], FT, tag="tp", name="tp")
                            for dx in range(3):
                                drrhs = dts[g][
                                    :, 2 * m + 1 : 2 * m + 5, dx : dx + 256
                                ].rearrange("p (a b) w -> p a b w", a=2)
                                nc.tensor.matmul(
                                    tp, lhsT=wdr_s[:, g, dx], rhs=drrhs,
                                    start=(dx == 0), stop=False, perf_mode=DRMODE,
                                )
                                nc.tensor.matmul(
                                    tp, lhsT=wn_s[:, g, dx],
                                    rhs=dts[g][:, 2 * m + 2 : 2 * m + 4, dx : dx + 256],
                                    start=False, stop=(dx == 2),
                                )
                            th = srng.tile([128, 2, W], BT, tag=f"th{g}", name=f"th{g}")
                            nc.scalar.activation(
                                th, tp, mybir.ActivationFunctionType.Tanh,
                                scale=float(2.0 ** (-SDE - SC2E)),
                            )
                            zt = srng.tile([128, 2, W], BT, tag=f"z{g}", name=f"z{g}")
                            nc.vector.scalar_tensor_tensor(
                                out=zt,
                                in0=dts[g][:, 2 * m + 2 : 2 * m + 4, 1:257],
                                scalar=float(2.0 ** (-SDE)),
                                in1=th, op0=AluOpType.mult, op1=AluOpType.add,
                            )
                            zts.append(zt)
                        g0 = srng.tile([128, 2, W], BT, tag="g0", name="g0")
                        nc.gpsimd.tensor_tensor(
                            out=g0, in0=zts[0], in1=zts[1], op=AluOpType.mult
                        )
                        z2b = srng.tile([42, 2, W], BT, tag="z2b", name="z2b")
                        nc.gpsimd.tensor_copy(z2b, zts[2][64:106])
                        g1 = srng.tile([42, 2, W], BT, tag="g1", name="g1")
                        nc.vector.tensor_tensor(
                            out=g1, in0=zts[2][0:42], in1=z2b, op=AluOpType.mult,
                        )
                        ro2 = ps_ro.tile([128, 2, W], FT, tag="ro", name="ro")
                        nc.tensor.matmul(
                            ro2[C:128], lhsT=wouta_s, rhs=g0, start=True, stop=False
                        )
                        nc.tensor.matmul(
                            ro2[C:128], lhsT=woutb_s, rhs=g1, start=False, stop=True
                        )
                        if m % 4 == 0:
                            ot_cur = ring.tile([C, 8, W], BT, tag="ot", name="ot")
                        nc.vector.tensor_tensor(
                            out=ot_cur[:, m % 4 * 2 : m % 4 * 2 + 2, :],
                            in0=ro2[C:128],
                            in1=x1t[:, 2 * m + 2 : 2 * m + 4, 1:257],
                            op=AluOpType.add,
                        )
                        if m % 4 == 3:
                            nc.sync.dma_start(
                                out=out[
                                    :, 64 * p + 2 * (m - 3) : 64 * p + 2 * (m - 3) + 8, :
                                ],
                                in_=ot_cur,
                            )

    nc.compile()
    return nc


# ---------------------------------------------------------------- host logic

_CACHE = {}


def _programs(affine):
    key = ("k", affine)
    if key not in _CACHE:
        _CACHE[key] = (_build_k1(affine), _build_k2(affine))
    return _CACHE[key]


def _diag_blocks(w, place):
    """w: [340] per-tap vector -> [3,128,128] diag matrices per placed group."""
    out = np.zeros((NG, 128, 128), F32)
    for s, ch in enumerate(place):
        if ch >= 0:
            out[s // 128, s % 128, s % 128] = w[ch]
    return out


def kernel(x, y, ln_w, ln_b, temperature, wq, wq_dw, wkv, wkv_dw, w_proj,
           w_in, w_dw, w_dw1, w_dw2, w_out):
    x = np.asarray(x, F32)
    y = np.asarray(y, F32)
    ln_w = np.asarray(ln_w, F32)
    ln_b = np.asarray(ln_b, F32)
    temperature = np.asarray(temperature, F32)
    wq = np.asarray(wq, F32)
    wq_dw = np.asarray(wq_dw, F32)
    wkv = np.asarray(wkv, F32)
    wkv_dw = np.asarray(wkv_dw, F32)
    w_proj = np.asarray(w_proj, F32)
    w_in = np.asarray(w_in, F32)
    w_dw = np.asarray(w_dw, F32)
    w_dw1 = np.asarray(w_dw1, F32)
    w_dw2 = np.asarray(w_dw2, F32)
    w_out = np.asarray(w_out, F32)

    affine = not (np.allclose(ln_w, 1.0) and np.allclose(ln_b, 0.0))
    k1, k2 = _programs(affine)

    # ---------- launch 1: q/k gram + norms + v
    xpad = np.zeros((B, C, H + 4, Wp), F32)
    xpad[:, :, 2 : 2 + H, 1 : 1 + W] = x
    ypad = np.zeros((B, C, H + 4, Wp), F32)
    ypad[:, :, 2 : 2 + H, 1 : 1 + W] = y

    def _fuse_pairs(w1x1, wdw):
        # w1x1: [O, C]; wdw: [O, 1, 3, 3] -> pairs [128, 3, O], singles [C, 3, O]
        O = w1x1.shape[0]
        pairs = np.zeros((128, 3, O), F32)
        sings = np.zeros((C, 3, O), F32)
        for p in range(3):
            pairs[0:C, p, :] = (w1x1 * wdw[:, 0, p, 0][:, None]).T
            pairs[C:128, p, :] = (w1x1 * wdw[:, 0, p, 2][:, None]).T
            sings[:, p, :] = (w1x1 * wdw[:, 0, p, 1][:, None]).T
        return pairs.astype(BF16), sings.astype(BF16)

    qpair, qsing = _fuse_pairs(wq, wq_dw)
    kvpair, kvsing = _fuse_pairs(wkv, wkv_dw)
    common1 = {
        "qpair": qpair,
        "qsing": qsing,
        "kvpair": kvpair,
        "kvsing": kvsing,
        "identb": np.eye(128).astype(BF16),
    }
    if affine:
        common1["gam"] = np.broadcast_to(ln_w[None, :], (128, C)).astype(BF16).copy()
        common1["bet"] = np.broadcast_to(ln_b[None, :], (128, C)).astype(BF16).copy()

    in_maps1 = []
    for core in range(NCORES):
        b, h = core // 2, core % 2
        rs = 2 + h * HS - 1  # padded-coords start row for halo-1 slab
        m = dict(common1)
        m["xh"] = np.ascontiguousarray(xpad[b, :, rs : rs + HS + 2, :]).astype(BF16)
        m["yh"] = np.ascontiguousarray(ypad[b, :, rs : rs + HS + 2, :]).astype(BF16)
        in_maps1.append(m)

    res1 = bass_utils.run_bass_kernel_spmd(k1, in_maps1, core_ids=list(range(NCORES)))

    # ---------- host combine: attention softmax -> P = w_proj @ blockdiag(A)
    pts = []
    vfull = np.zeros((B, C, H, W), BF16)
    for b in range(B):
        r0, r1 = res1.results[2 * b], res1.results[2 * b + 1]
        G = r0["gramo"].astype(np.float64).sum(1) + r1["gramo"].astype(np.float64).sum(1)
        qss = r0["qsso"].astype(np.float64).sum(1) + r1["qsso"].astype(np.float64).sum(1)
        kss = r0["ksso"].astype(np.float64).sum(1) + r1["ksso"].astype(np.float64).sum(1)
        nq = np.maximum(np.sqrt(qss), 1e-12)
        nk = np.maximum(np.sqrt(kss), 1e-12)
        A = np.zeros((C, C), np.float64)
        for hh in range(HEADS):
            sl = slice(hh * CH, (hh + 1) * CH)
            logits = temperature[hh, 0, 0] * (G[sl, sl] / np.outer(nq[sl], nk[sl]))
            e = np.exp(logits - logits.max(axis=-1, keepdims=True))
            A[sl, sl] = e / e.sum(axis=-1, keepdims=True)
        P = w_proj.astype(np.float64) @ A
        pts.append(np.ascontiguousarray(P.T).astype(BF16))
        vfull[b, :, 0:HS] = r0["vout"]
        vfull[b, :, HS:H] = r1["vout"]

    # ---------- launch 2: x1 = x + P v ; IEL (v2: fp8 DoubleRow kernel)
    vpad = np.zeros((B, C, H + 4, W), BF16)
    vpad[:, :, 2 : 2 + H, :] = vfull
    xpad16 = xpad.astype(BF16)

    w_in_p = np.zeros((NG * 128, C), F32)
    w_dw_p = np.zeros((NG * 128, 3, 3), F32)
    w12 = np.concatenate([w_dw1[:, 0], w_dw2[:, 0]], axis=0)  # [340,3,3]
    w12_p = np.zeros((NG * 128, 3, 3), F32)
    for s, ch in enumerate(PLACE340):
        if ch >= 0:
            w_in_p[s] = w_in[ch]
            w_dw_p[s] = w_dw[ch, 0]
            w12_p[s] = w12[ch]
    SC1 = float(2.0 ** SC1E)
    SC2 = float(2.0 ** SC2E)
    # fused-d weights: DR pairs (dy=-1,+1), DR singles, norm pair/single (dy=0)
    fdrp = np.zeros((128, NG, 2, 128), F32)
    fdrs = np.zeros((C, NG, 2, 128), F32)
    fnp = np.zeros((128, NG, 128), F32)
    fns = np.zeros((C, NG, 128), F32)
    for g in range(NG):
        sl = slice(g * 128, (g + 1) * 128)
        wi = w_in_p[sl]  # [128m, 64c]
        wd = w_dw_p[sl]  # [128m, 3, 3]
        for t, dy in enumerate((0, 2)):  # tap rows: dy=-1 -> 0, dy=+1 -> 2
            fdrp[0:C, g, t, :] = (wi * wd[:, dy, 0][:, None]).T  # dx=-1 half
            fdrp[C:128, g, t, :] = (wi * wd[:, dy, 2][:, None]).T  # dx=+1 half
            fdrs[:, g, t, :] = (wi * wd[:, dy, 1][:, None]).T
        fnp[0:C, g, :] = (wi * wd[:, 1, 0][:, None]).T
        fnp[C:128, g, :] = (wi * wd[:, 1, 2][:, None]).T
        fns[:, g, :] = (wi * wd[:, 1, 1][:, None]).T
    # dw12 diag weights
    wdr = np.zeros((128, NG, 3, 2, 128), F32)
    wn = np.zeros((128, NG, 3, 128), F32)
    for g in range(NG):
        sl = slice(g * 128, (g + 1) * 128)
        for dx in range(3):
            for t, dy in enumerate((0, 2)):
                wdr[np.arange(128), g, dx, t, np.arange(128)] = w12_p[sl, dy, dx]
            wn[np.arange(128), g, dx, np.arange(128)] = w12_p[sl, 1, dx]
    vsel = np.zeros((C, BAND, C), F32)
    for j in range(BAND):
        vsel[:, j, j] = 1.0 / 64.0
    bsel = np.zeros((BAND, BAND, C), F32)
    for j in range(BAND):
        bsel[j, j, :] = 1.0

    common2 = {
        "vsel": vsel.astype(BF16),
        "bsel": bsel.astype(BF16),
        "fdrp": (fdrp * SC1).astype(E4M3),
        "fdrs": (fdrs * SC1).astype(E4M3),
        "fnp": (fnp * SC1).astype(E4M3),
        "fns": (fns * SC1).astype(E4M3),
        "wdr": (wdr * SC2).astype(E4M3),
        "wn": (wn * SC2).astype(E4M3),
        "wouta": np.ascontiguousarray(w_out.T[0:128]).astype(BF16),
        "woutb": np.ascontiguousarray(w_out.T[128:170]).astype(BF16),
    }
    if affine:
        common2["gamv"] = ln_w[:, None].astype(F32)
        common2["betv"] = ln_b[:, None].astype(F32)

    in_maps2 = []
    for core in range(NCORES):
        b, h = core // 2, core % 2
        rs = h * HS  # padded-coords start row (halo-2 slab of 132 rows)
        m = dict(common2)
        m["xk"] = np.ascontiguousarray(xpad16[b, :, rs : rs + HS + 4, :])
        m["vk"] = np.ascontiguousarray(vpad[b, :, rs : rs + HS + 4, :])
        m["ptw"] = pts[b]
        in_maps2.append(m)

    res2 = bass_utils.run_bass_kernel_spmd(k2, in_maps2, core_ids=list(range(NCORES)))

    out = np.zeros((B, C, H, W), F32)
    for core in range(NCORES):
        b, h = core // 2, core % 2
        out[b, :, h * HS : (h + 1) * HS, :] = res2.results[core]["out"].astype(F32)
    return out



# revision 9
# speedup vs baseline: 1.7248x; 1.1398x over previous
"""Trainium2 Bass kernel for nn_CSDC_8246337208509 (I_LCA block: CAB cross-attention + IEL gated FFN).

Contract: kernel(**inputs) takes FULL unsharded inputs, returns FULL output.
Sharding: 8 cores = 4 batches x 2 spatial halves (128 rows of H each).
Two device launches with a tiny host-side combine (attention softmax over 8x8
per-head Gram matrices) between them.
"""

import contextlib
import sys

import numpy as np

try:
    import concourse.bass as bass  # noqa: F401
except Exception:  # pragma: no cover
    sys.path.insert(0, "/opt/trn_rl_repo")
    sys.path.insert(0, "/root/.axon_site/_ro/trn_rl_repo")

import concourse.bacc as bacc
import concourse.tile as tile
from concourse import mybir
from concourse import bass_utils
from concourse.alu_op_type import AluOpType
import ml_dtypes

BF16 = ml_dtypes.bfloat16
F32 = np.float32
BT = mybir.dt.bfloat16
FT = mybir.dt.float32

B, C, H, W = 4, 64, 256, 256
HEADS, CH = 8, 8
HID = 170
EPS = 1e-6
Wp = W + 2  # 258, zero col at 0 and 257
HS = H // 2  # 128 interior rows per core
Hb1 = 16  # k1 band interior rows
NB1 = HS // Hb1
Hb2 = 8  # k2 band interior rows
NB2 = HS // Hb2
NCORES = 8

TAPS = [(ty - 1, tx - 1) for ty in range(3) for tx in range(3)]  # (dy, dx), t = ty*3+tx

F8 = mybir.dt.float8e4
E4M3 = ml_dtypes.float8_e4m3
Wf = 272  # fp8 padded row stride (mult of 16)
SC1E = 10  # fused-d weight scale exponent
SDE = 5    # fp8 d storage scale exponent
SC2E = 9   # dw12 weight scale exponent
PH = 68    # k2 pass tile height
NCL = 34
NCD = 33
NCO = 32
BAND = 16
NBAND = 3
DRMODE = mybir.MatmulPerfMode.DoubleRow
PH1 = 130
NCL1 = 65
NCQ = 64
NB1K = 5



# channel placement for the 340-wide IEL stream into 3 groups of 128 partitions:
# G0 = x1[0:128]; G1 = x2[0:128]; G2: slots 0..41 = x1[128:170], slots 64..105 =
# x2[128:170] (partition bases must be 32-aligned, so the x2 tail sits at 64).
PLACE340 = [-1] * 384
for _i in range(128):
    PLACE340[_i] = _i           # G0
    PLACE340[128 + _i] = 170 + _i  # G1
for _i in range(42):
    PLACE340[256 + _i] = 128 + _i  # G2 low: x1 tail
    PLACE340[256 + 64 + _i] = 298 + _i  # G2 high: x2 tail
NG = 3


# ---------------------------------------------------------------- device code

def _ln_into(nc, tc, pools, src, nrows, dst, affine, src_f32=False):
    """Channels-first LayerNorm of src[:, :nrows, 1:257] -> dst (S-stacked bf16).

    src: [64, nrows, 258] bf16 tile. dst: [128, nrows, 260] S-layout tile whose
    pad cols are already zeroed: top half dst[0:64, r, c] = ln[r, c-1] (written
    at cols 2:258), bottom half dst[64:128, r, c] = ln[r, c+1] (gpsimd copy).
    All transposes ride the DMA xbar (bf16), not the PE.
    """
    lnscr = pools["lnscr"]
    T = nrows * 2
    xTs = lnscr.tile([128, T, 64], BT, tag="ln_xTs")
    xnT = lnscr.tile([128, T, 64], BT, tag="ln_xnT")
    st = lnscr.tile([128, T, 6], FT, tag="ln_st")
    mv = lnscr.tile([128, T, 2], FT, tag="ln_mv")
    sr = lnscr.tile([128, T, 1], FT, tag="ln_sr")
    ri = lnscr.tile([128, T, 1], FT, tag="ln_ri")

    ps_t = pools["ps_t"]
    ident = pools["idf"] if src_f32 else pools["idb"]
    for g in range((T + 7) // 8):
        n = min(8, T - g * 8)
        pt = ps_t.tile([128, 8, 64], FT if src_f32 else BT, tag="ps_fw")
        for j in range(n):
            t = g * 8 + j
            row, half = t // 2, t % 2
            nc.tensor.transpose(
                pt[:, j, :],
                src[:, row, 1 + 128 * half : 1 + 128 * half + 128],
                ident[0:64, 0:64],
            )
        (nc.scalar.copy if g % 2 == 0 else nc.vector.tensor_copy)(
            xTs[:, g * 8 : g * 8 + n, :], pt[:, 0:n, :]
        )
    for t in range(T):
        nc.vector.bn_stats(st[:, t, :], xTs[:, t, :])
        nc.vector.bn_aggr(mv[:, t, :], st[:, t, :])
    nc.scalar.activation(sr, mv[:, :, 1:2], mybir.ActivationFunctionType.Sqrt, bias=pools["eps"])
    nc.vector.reciprocal(ri, sr)
    for t in range(T):
        nc.vector.tensor_scalar(
            out=xnT[:, t, :],
            in0=xTs[:, t, :],
            scalar1=mv[:, t, 0:1],
            scalar2=ri[:, t, 0:1],
            op0=AluOpType.subtract,
            op1=AluOpType.mult,
        )
    if affine:
        gam_bc, bet_bc = pools["gam_bc"], pools["bet_bc"]
        for t in range(T):
            nc.vector.tensor_tensor(out=xnT[:, t, :], in0=xnT[:, t, :], in1=gam_bc, op=AluOpType.mult)
            nc.vector.tensor_tensor(out=xnT[:, t, :], in0=xnT[:, t, :], in1=bet_bc, op=AluOpType.add)
    # transpose back (PE) into the S-layout top half, then gpsimd-fill the bottom
    for g in range((T + 3) // 4):
        pb = ps_t.tile([128, 2, 256], BT, tag="ps_bw")
        for j in range(4):
            t = g * 4 + j
            nc.tensor.transpose(
                pb[0:64, j // 2, 128 * (j % 2) : 128 * (j % 2) + 128],
                xnT[:, t, :],
                pools["idb"],
            )
        (nc.scalar.copy if g % 2 == 0 else nc.vector.tensor_copy)(
            dst[0:64, g * 2 : g * 2 + 2, 2:258], pb[0:64]
        )
        nc.gpsimd.tensor_copy(
            dst[64:128, g * 2 : g * 2 + 2, 0:256], dst[0:64, g * 2 : g * 2 + 2, 2:258]
        )


def _zero_pad_cols(nc, t, nrows):
    nc.gpsimd.memset(t[:, 0:nrows, 0:1], 0.0)
    nc.gpsimd.memset(t[:, 0:nrows, 257:258], 0.0)


def _zero_pad_cols_s(nc, t, nrows):
    # S-stacked layout [128, nrows, 260]: top half holds u[c-1], bottom u[c+1]
    nc.gpsimd.memset(t[0:64, 0:nrows, 0:2], 0.0)
    nc.gpsimd.memset(t[0:64, 0:nrows, 258:260], 0.0)
    nc.gpsimd.memset(t[64:128, 0:nrows, 256:260], 0.0)


# fused conv1x1+dw3x3: 3 K=128 pair-matmuls + 3 K=64 single-matmuls per chunk.
# S: [128, nr, 260] stacked input; out rows j correspond to S rows j+roff.
def _fused_conv(nc, ps_pool, pairs, sings, S, roff, nchunks, evict, M=128):
    for c in range(nchunks):
        pt = ps_pool.tile([128, 2, W], FT, tag="ps_mm")
        for p in range(3):
            dy = p - 1
            nc.tensor.matmul(
                pt[0:M],
                lhsT=pairs[:, p, :],
                rhs=S[:, roff + 2 * c + dy : roff + 2 * c + dy + 2, 1:257],
                start=(p == 0),
                stop=False,
            )
        for i in range(3):
            dy = i - 1
            nc.tensor.matmul(
                pt[0:M],
                lhsT=sings[:, i, :],
                rhs=S[0:64, roff + 2 * c + dy : roff + 2 * c + dy + 2, 2:258],
                start=False,
                stop=(i == 2),
            )
        evict(c, pt[0:M])


def _build_k1(affine):
    nc = bacc.Bacc("TRN2", target_bir_lowering=False, debug=False)
    xh = nc.dram_tensor("xh", [C, PH1, Wp], BT, kind="ExternalInput").ap()
    yh = nc.dram_tensor("yh", [C, PH1, Wp], BT, kind="ExternalInput").ap()
    qdrp = nc.dram_tensor("qdrp", [128, 2, C], F8, kind="ExternalInput").ap()
    qdrs = nc.dram_tensor("qdrs", [C, 2, C], F8, kind="ExternalInput").ap()
    qnp = nc.dram_tensor("qnp", [128, C], F8, kind="ExternalInput").ap()
    qns = nc.dram_tensor("qns", [C, C], F8, kind="ExternalInput").ap()
    kdrp = nc.dram_tensor("kdrp", [128, 2, 2 * C], F8, kind="ExternalInput").ap()
    kdrs = nc.dram_tensor("kdrs", [C, 2, 2 * C], F8, kind="ExternalInput").ap()
    knp = nc.dram_tensor("knp", [128, 2 * C], F8, kind="ExternalInput").ap()
    kns = nc.dram_tensor("kns", [C, 2 * C], F8, kind="ExternalInput").ap()
    vsel = nc.dram_tensor("vsel", [C, BAND, C], BT, kind="ExternalInput").ap()
    bsel = nc.dram_tensor("bsel", [BAND, BAND, C], BT, kind="ExternalInput").ap()
    identb = nc.dram_tensor("identb", [128, 128], BT, kind="ExternalInput").ap()
    if affine:
        gamv = nc.dram_tensor("gamv", [C, 1], FT, kind="ExternalInput").ap()
        betv = nc.dram_tensor("betv", [C, 1], FT, kind="ExternalInput").ap()

    qg = nc.dram_tensor("qg", [128, 128], FT, kind="ExternalOutput").ap()
    vout = nc.dram_tensor("vout", [C, 2 * C, W], BT, kind="ExternalOutput").ap()

    with tile.TileContext(nc) as tc:
        with contextlib.ExitStack() as ctx:
            wp = ctx.enter_context(tc.tile_pool(name="wp", bufs=1))
            big = ctx.enter_context(tc.tile_pool(name="big", bufs=1))
            ring = ctx.enter_context(tc.tile_pool(name="ring", bufs=3))
            srng = ctx.enter_context(tc.tile_pool(name="srng", bufs=2))
            ps_ub = ctx.enter_context(tc.tile_pool(name="ps_ub", bufs=2, space="PSUM"))
            ps_v = ctx.enter_context(tc.tile_pool(name="ps_v", bufs=1, space="PSUM"))
            ps_r = ctx.enter_context(tc.tile_pool(name="ps_r", bufs=1, space="PSUM"))
            ps_cv = ctx.enter_context(tc.tile_pool(name="ps_cv", bufs=2, space="PSUM"))
            ps_tp = ctx.enter_context(tc.tile_pool(name="ps_tp", bufs=1, space="PSUM"))
            ps_g = ctx.enter_context(tc.tile_pool(name="ps_g", bufs=1, space="PSUM"))

            qdrp_s = wp.tile([128, 2, C], F8)
            nc.sync.dma_start(out=qdrp_s, in_=qdrp)
            qdrs_s = wp.tile([C, 2, C], F8)
            nc.sync.dma_start(out=qdrs_s, in_=qdrs)
            qnp_s = wp.tile([128, C], F8)
            nc.sync.dma_start(out=qnp_s, in_=qnp)
            qns_s = wp.tile([C, C], F8)
            nc.sync.dma_start(out=qns_s, in_=qns)
            kdrp_s = wp.tile([128, 2, 2 * C], F8)
            nc.sync.dma_start(out=kdrp_s, in_=kdrp)
            kdrs_s = wp.tile([C, 2, 2 * C], F8)
            nc.sync.dma_start(out=kdrs_s, in_=kdrs)
            knp_s = wp.tile([128, 2 * C], F8)
            nc.sync.dma_start(out=knp_s, in_=knp)
            kns_s = wp.tile([C, 2 * C], F8)
            nc.sync.dma_start(out=kns_s, in_=kns)
            vsel_s = wp.tile([C, BAND, C], BT)
            nc.sync.dma_start(out=vsel_s, in_=vsel)
            bsel_s = wp.tile([BAND, BAND, C], BT)
            nc.sync.dma_start(out=bsel_s, in_=bsel)
            id_s = wp.tile([128, 128], BT)
            nc.sync.dma_start(out=id_s, in_=identb)
            usel_s = wp.tile([C, C], BT)
            nc.vector.memset(usel_s, 1.0 / 64.0)
            eps_s = wp.tile([128, 1], FT)
            nc.vector.memset(eps_s, EPS)
            if affine:
                gam_s = wp.tile([C, 1], FT)
                nc.sync.dma_start(out=gam_s, in_=gamv)
                bet_s = wp.tile([C, 1], FT)
                nc.sync.dma_start(out=bet_s, in_=betv)

            lnSx = big.tile([128, PH1, Wf], F8, name="lnSx")
            lnSy = big.tile([128, PH1, Wf], F8, name="lnSy")
            xy = big.tile([128, PH1, W], BT, name="xy")  # top xc, bottom yc
            rbx = big.tile([BAND, NB1, 512], BT, name="rbx")
            rby = big.tile([BAND, NB1, 512], BT, name="rby")

            for S in (lnSx, lnSy):
                nc.gpsimd.memset(S[0:C, :, 0:2], 0.0)
                nc.gpsimd.memset(S[0:C, :, 258:Wf], 0.0)
                nc.gpsimd.memset(S[C:128, :, 256:Wf], 0.0)

            gp = ps_g.tile([128, 128], FT, name="gp")

            nrow_grp = [8] * 16 + [2]
            xbs = ybs = None
            vps_cur = None
            vt_cur = None

            for i in range(NCQ + 22):
                # ---------------- stage 1: moments
                c = i
                if c < NCL1:
                    if c % 4 == 0:
                        g4 = c // 4
                        nr = nrow_grp[g4]
                        xb = ring.tile([C, 8, Wp], BT, tag="xb", name="xb")
                        nc.sync.dma_start(
                            out=xb[:, 0:nr, :], in_=xh[:, 8 * g4 : 8 * g4 + nr, :]
                        )
                        yb = ring.tile([C, 8, Wp], BT, tag="yb", name="yb")
                        nc.sync.dma_start(
                            out=yb[:, 0:nr, :], in_=yh[:, 8 * g4 : 8 * g4 + nr, :]
                        )
                        xbs, ybs = xb, yb
                    lr = c % 4 * 2
                    ubt = ps_ub.tile([128, 2, W], FT, tag="ubt", name="ubt")
                    nc.tensor.matmul(
                        ubt[0:C], lhsT=usel_s, rhs=xbs[:, lr : lr + 2, 1:257],
                        start=True, stop=True,
                    )
                    nc.tensor.matmul(
                        ubt[C:128], lhsT=usel_s, rhs=ybs[:, lr : lr + 2, 1:257],
                        start=True, stop=True,
                    )
                    nc.vector.tensor_tensor(
                        out=xy[0:C, 2 * c : 2 * c + 2, :],
                        in0=xbs[:, lr : lr + 2, 1:257], in1=ubt[0:C],
                        op=AluOpType.subtract,
                    )
                    nc.vector.tensor_tensor(
                        out=xy[C:128, 2 * c : 2 * c + 2, :],
                        in0=ybs[:, lr : lr + 2, 1:257], in1=ubt[C:128],
                        op=AluOpType.subtract,
                    )
                    xq = srng.tile([C, 2, W], BT, tag="xq", name="xq")
                    nc.vector.tensor_tensor(
                        out=xq, in0=xy[0:C, 2 * c : 2 * c + 2, :],
                        in1=xy[0:C, 2 * c : 2 * c + 2, :], op=AluOpType.mult,
                    )
                    yq = srng.tile([C, 2, W], BT, tag="yq", name="yq")
                    nc.vector.tensor_tensor(
                        out=yq, in0=xy[C:128, 2 * c : 2 * c + 2, :],
                        in1=xy[C:128, 2 * c : 2 * c + 2, :], op=AluOpType.mult,
                    )
                    B = c // BAND
                    j = c % BAND
                    if j == 0:
                        vps_cur = ps_v.tile([128, 512], FT, tag="vps", name="vps")
                    last = j == BAND - 1 or c == NCL1 - 1
                    nc.tensor.matmul(
                        vps_cur[0:C], lhsT=vsel_s[:, j, :], rhs=xq,
                        start=(j == 0), stop=last,
                    )
                    nc.tensor.matmul(
                        vps_cur[C:128], lhsT=vsel_s[:, j, :], rhs=yq,
                        start=(j == 0), stop=last,
                    )
                    if last:
                        sbx = srng.tile([BAND, 512], BT, tag="sbx", name="sbx")
                        nc.scalar.activation(
                            sbx, vps_cur[0:BAND],
                            mybir.ActivationFunctionType.Sqrt, bias=eps_s[0:BAND],
                        )
                        sby = srng.tile([BAND, 512], BT, tag="sby", name="sby")
                        nc.scalar.activation(
                            sby, vps_cur[C : C + BAND],
                            mybir.ActivationFunctionType.Sqrt, bias=eps_s[0:BAND],
                        )
                        with nc.allow_low_precision(reason="bf16 rsqrt rows"):
                            nc.vector.reciprocal(rbx[:, B, :], sbx)
                            nc.vector.reciprocal(rby[:, B, :], sby)

                # ---------------- stage 2: apply (lag 16)
                jc = i - BAND
                if 0 <= jc < NCL1:
                    B = jc // BAND
                    jj = jc % BAND
                    rbc = ps_r.tile([128, 2, W], FT, tag="rbc", name="rbc")
                    nc.tensor.matmul(
                        rbc[0:C], lhsT=bsel_s[:, jj, :], rhs=rbx[:, B, :],
                        start=True, stop=True,
                    )
                    nc.tensor.matmul(
                        rbc[C:128], lhsT=bsel_s[:, jj, :], rhs=rby[:, B, :],
                        start=True, stop=True,
                    )
                    for S, half in ((lnSx, 0), (lnSy, 1)):
                        src = xy[half * C : half * C + C, 2 * jc : 2 * jc + 2, :]
                        if affine:
                            tmp = srng.tile([C, 2, W], BT, tag=f"tmp{half}", name=f"tmp{half}")
                            nc.vector.tensor_tensor(
                                out=tmp, in0=src,
                                in1=rbc[half * C : half * C + C],
                                op=AluOpType.mult,
                            )
                            nc.vector.tensor_scalar(
                                out=S[0:C, 2 * jc : 2 * jc + 2, 2:258],
                                in0=tmp, scalar1=gam_s, scalar2=bet_s,
                                op0=AluOpType.mult, op1=AluOpType.add,
                            )
                        else:
                            nc.vector.tensor_tensor(
                                out=S[0:C, 2 * jc : 2 * jc + 2, 2:258],
                                in0=src, in1=rbc[half * C : half * C + C],
                                op=AluOpType.mult,
                            )
                        nc.gpsimd.tensor_copy(
                            S[C:128, 2 * jc : 2 * jc + 2, 0:256],
                            S[0:C, 2 * jc : 2 * jc + 2, 2:258],
                        )

                # ---------------- stage 3: q/kv conv, evicts, transposes (lag 18)
                m = i - BAND - 2
                if 0 <= m < NCQ:
                    qk = srng.tile([128, 2, W], BT, tag="qk", name="qk", bufs=3)
                    for S, drp, drs, np_, ns_, MM in (
                        (lnSx, qdrp_s, qdrs_s, qnp_s, qns_s, C),
                        (lnSy, kdrp_s, kdrs_s, knp_s, kns_s, 2 * C),
                    ):
                        cv = ps_cv.tile([128, 2, W], FT, tag="cv", name="cv")
                        prhs = S[:, 2 * m : 2 * m + 4, 1:257].rearrange(
                            "p (a b) w -> p a b w", a=2
                        )
                        srhs = S[0:C, 2 * m : 2 * m + 4, 2:258].rearrange(
                            "p (a b) w -> p a b w", a=2
                        )
                        nc.tensor.matmul(
                            cv[0:MM], lhsT=drp, rhs=prhs,
                            start=True, stop=False, perf_mode=DRMODE,
                        )
                        nc.tensor.matmul(
                            cv[0:MM], lhsT=drs, rhs=srhs,
                            start=False, stop=False, perf_mode=DRMODE,
                        )
                        nc.tensor.matmul(
                            cv[0:MM], lhsT=np_,
                            rhs=S[:, 2 * m + 1 : 2 * m + 3, 1:257],
                            start=False, stop=False,
                        )
                        nc.tensor.matmul(
                            cv[0:MM], lhsT=ns_,
                            rhs=S[0:C, 2 * m + 1 : 2 * m + 3, 2:258],
                            start=False, stop=True,
                        )
                        if MM == C:
                            nc.scalar.activation(
                                qk[0:C], cv[0:C],
                                mybir.ActivationFunctionType.Copy,
                                scale=float(2.0 ** (-SC1E)),
                            )
                        else:
                            nc.scalar.activation(
                                qk[C:128], cv[0:C],
                                mybir.ActivationFunctionType.Copy,
                                scale=float(2.0 ** (-SC1E)),
                            )
                            if m % 4 == 0:
                                vt_cur = ring.tile([C, 8, W], BT, tag="vt", name="vt")
                            nc.scalar.activation(
                                vt_cur[:, m % 4 * 2 : m % 4 * 2 + 2, :],
                                cv[C : 2 * C],
                                mybir.ActivationFunctionType.Copy,
                                scale=float(2.0 ** (-SC1E)),
                            )
                            if m % 4 == 3:
                                nc.sync.dma_start(
                                    out=vout[:, 2 * (m - 3) : 2 * (m - 3) + 8, :],
                                    in_=vt_cur,
                                )
                    tp = ps_tp.tile([128, 4, 128], BT, tag="tp", name="tp")
                    for b in range(4):
                        nc.tensor.transpose(
                            tp[:, b, :], qk[:, b // 2, 128 * (b % 2) : 128 * (b % 2) + 128],
                            id_s,
                        )
                    qkT = srng.tile([128, 4, 128], BT, tag="qkT", name="qkT", bufs=3)
                    nc.vector.tensor_copy(qkT, tp)
                    for b in range(4):
                        nc.tensor.matmul(
                            gp, lhsT=qkT[:, b, :], rhs=qkT[:, b, :],
                            start=(m == 0 and b == 0), stop=(m == NCQ - 1 and b == 3),
                        )

            gsb = wp.tile([128, 128], FT)
            nc.vector.tensor_copy(gsb, gp)
            nc.sync.dma_start(out=qg, in_=gsb)

    nc.compile()
    return nc


def _build_k2(affine):
    nc = bacc.Bacc("TRN2", target_bir_lowering=False, debug=False)
    xk = nc.dram_tensor("xk", [C, 132, Wp], BT, kind="ExternalInput").ap()
    vk = nc.dram_tensor("vk", [C, 132, W], BT, kind="ExternalInput").ap()
    ptw = nc.dram_tensor("ptw", [C, C], BT, kind="ExternalInput").ap()
    vsel = nc.dram_tensor("vsel", [C, BAND, C], BT, kind="ExternalInput").ap()
    bsel = nc.dram_tensor("bsel", [BAND, BAND, C], BT, kind="ExternalInput").ap()
    fdrp = nc.dram_tensor("fdrp", [128, NG, 2, 128], F8, kind="ExternalInput").ap()
    fdrs = nc.dram_tensor("fdrs", [C, NG, 2, 128], F8, kind="ExternalInput").ap()
    fnp = nc.dram_tensor("fnp", [128, NG, 128], F8, kind="ExternalInput").ap()
    fns = nc.dram_tensor("fns", [C, NG, 128], F8, kind="ExternalInput").ap()
    wdr = nc.dram_tensor("wdr", [128, NG, 3, 2, 128], F8, kind="ExternalInput").ap()
    wn = nc.dram_tensor("wn", [128, NG, 3, 128], F8, kind="ExternalInput").ap()
    wouta = nc.dram_tensor("wouta", [128, C], BT, kind="ExternalInput").ap()
    woutb = nc.dram_tensor("woutb", [42, C], BT, kind="ExternalInput").ap()
    if affine:
        gamv = nc.dram_tensor("gamv", [C, 1], FT, kind="ExternalInput").ap()
        betv = nc.dram_tensor("betv", [C, 1], FT, kind="ExternalInput").ap()

    out = nc.dram_tensor("out", [C, HS, W], BT, kind="ExternalOutput").ap()

    with tile.TileContext(nc) as tc:
        with contextlib.ExitStack() as ctx:
            wp = ctx.enter_context(tc.tile_pool(name="wp", bufs=1))
            big = ctx.enter_context(tc.tile_pool(name="big", bufs=1))
            ring = ctx.enter_context(tc.tile_pool(name="ring", bufs=3))
            srng = ctx.enter_context(tc.tile_pool(name="srng", bufs=2))
            ps_pu = ctx.enter_context(tc.tile_pool(name="ps_pu", bufs=2, space="PSUM"))
            ps_ro = ctx.enter_context(tc.tile_pool(name="ps_ro", bufs=2, space="PSUM"))
            ps_var = ctx.enter_context(tc.tile_pool(name="ps_var", bufs=1, space="PSUM"))
            ps_d = ctx.enter_context(tc.tile_pool(name="ps_d", bufs=2, space="PSUM"))
            ps_t = ctx.enter_context(tc.tile_pool(name="ps_t", bufs=1, space="PSUM"))

            # ------------------------------------------------ persistent weights
            ptw_s = wp.tile([C, C], BT)
            nc.sync.dma_start(out=ptw_s, in_=ptw)
            vsel_s = wp.tile([C, BAND, C], BT)
            nc.sync.dma_start(out=vsel_s, in_=vsel)
            bsel_s = wp.tile([BAND, BAND, C], BT)
            nc.sync.dma_start(out=bsel_s, in_=bsel)
            fdrp_s = wp.tile([128, NG, 2, 128], F8)
            nc.sync.dma_start(out=fdrp_s, in_=fdrp)
            fdrs_s = wp.tile([C, NG, 2, 128], F8)
            nc.sync.dma_start(out=fdrs_s, in_=fdrs)
            fnp_s = wp.tile([128, NG, 128], F8)
            nc.sync.dma_start(out=fnp_s, in_=fnp)
            fns_s = wp.tile([C, NG, 128], F8)
            nc.sync.dma_start(out=fns_s, in_=fns)
            wdr_s = wp.tile([128, NG, 3, 2, 128], F8)
            nc.sync.dma_start(out=wdr_s, in_=wdr)
            wn_s = wp.tile([128, NG, 3, 128], F8)
            nc.sync.dma_start(out=wn_s, in_=wn)
            wouta_s = wp.tile([128, C], BT)
            nc.sync.dma_start(out=wouta_s, in_=wouta)
            woutb_s = wp.tile([42, C], BT)
            nc.sync.dma_start(out=woutb_s, in_=woutb)
            usel_s = wp.tile([C, C], BT)
            nc.vector.memset(usel_s, 1.0 / 64.0)
            ones1 = wp.tile([1, C], BT)
            nc.vector.memset(ones1, 1.0)
            eps_s = wp.tile([128, 1], FT)
            nc.vector.memset(eps_s, EPS)
            if affine:
                gam_s = wp.tile([C, 1], FT)
                nc.sync.dma_start(out=gam_s, in_=gamv)
                bet_s = wp.tile([C, 1], FT)
                nc.sync.dma_start(out=bet_s, in_=betv)

            for p in range(2):
                d0 = 64 * p  # dram row of local row 0

                xt = big.tile([128, PH, Wp], BT, tag="x1", name=f"x1_{p}")
                x1t = xt[0:C]
                xcs = xt[C:128, :, 1:257]
                lnS = big.tile([128, PH, Wf], F8, tag="lnS", name=f"lnS_{p}")
                dts = [
                    big.tile([128, PH, Wf], F8, tag=f"d{g}", name=f"d{g}_{p}")
                    for g in range(NG)
                ]
                rband = big.tile([BAND, NBAND, 512], BT, tag="rband", name=f"rband_{p}")

                nc.gpsimd.memset(lnS[0:C, :, 0:2], 0.0)
                nc.gpsimd.memset(lnS[0:C, :, 258:Wf], 0.0)
                nc.gpsimd.memset(lnS[C:128, :, 256:Wf], 0.0)
                for g in range(NG):
                    nc.gpsimd.memset(dts[g][:, :, 0:1], 0.0)
                    nc.gpsimd.memset(dts[g][:, :, 257:Wf], 0.0)

                nrow_grp = [8] * 8 + [4]
                xbs = vbs = None
                vps_cur = None
                ot_cur = None

                for i in range(NCO + 20):
                    # ---------------- stage 1
                    c = i
                    if c < NCL:
                        if c % 4 == 0:
                            g4 = c // 4
                            nr = nrow_grp[g4]
                            xb = ring.tile([C, 8, Wp], BT, tag="xb", name="xb")
                            nc.sync.dma_start(
                                out=xb[:, 0:nr, :],
                                in_=xk[:, d0 + 8 * g4 : d0 + 8 * g4 + nr, :],
                            )
                            vb = ring.tile([C, 8, W], BT, tag="vb", name="vb")
                            nc.sync.dma_start(
                                out=vb[:, 0:nr, :],
                                in_=vk[:, d0 + 8 * g4 : d0 + 8 * g4 + nr, :],
                            )
                            xbs, vbs = xb, vb
                        lr = c % 4 * 2
                        pu = ps_pu.tile([128, 2, W], FT, tag="pu", name="pu")
                        nc.tensor.matmul(
                            pu[0:C], lhsT=ptw_s, rhs=vbs[:, lr : lr + 2, :],
                            start=True, stop=True,
                        )
                        nc.vector.tensor_tensor(
                            out=x1t[:, 2 * c : 2 * c + 2, 1:257],
                            in0=pu[0:C],
                            in1=xbs[:, lr : lr + 2, 1:257],
                            op=AluOpType.add,
                        )
                        nc.tensor.matmul(
                            pu[C:128], lhsT=usel_s,
                            rhs=x1t[:, 2 * c : 2 * c + 2, 1:257],
                            start=True, stop=True,
                        )
                        nc.vector.tensor_tensor(
                            out=xcs[:, 2 * c : 2 * c + 2, :],
                            in0=x1t[:, 2 * c : 2 * c + 2, 1:257],
                            in1=pu[C:128],
                            op=AluOpType.subtract,
                        )
                        xq = srng.tile([C, 2, W], BT, tag="xq", name="xq")
                        nc.vector.tensor_tensor(
                            out=xq, in0=xcs[:, 2 * c : 2 * c + 2, :],
                            in1=xcs[:, 2 * c : 2 * c + 2, :], op=AluOpType.mult,
                        )
                        B = c // BAND
                        j = c % BAND
                        if j == 0:
                            vps_cur = ps_var.tile([C, 512], FT, tag="vps", name="vps")
                        nc.tensor.matmul(
                            vps_cur, lhsT=vsel_s[:, j, :], rhs=xq,
                            start=(j == 0), stop=(j == BAND - 1 or c == NCL - 1),
                        )
                        if j == BAND - 1 or c == NCL - 1:
                            sb = srng.tile([BAND, 512], BT, tag="sb", name="sb")
                            nc.scalar.activation(
                                sb, vps_cur[0:BAND], mybir.ActivationFunctionType.Sqrt,
                                bias=eps_s[0:BAND],
                            )
                            with nc.allow_low_precision(reason="bf16 rsqrt rows"):
                                nc.vector.reciprocal(rband[:, B, :], sb)

                    # ---------------- stage 2: apply (lag 16)
                    jc = i - BAND
                    if 0 <= jc < NCL:
                        B = jc // BAND
                        jj = jc % BAND
                        ro = ps_ro.tile([128, 2, W], FT, tag="ro", name="ro")
                        nc.tensor.matmul(
                            ro[0:C], lhsT=bsel_s[:, jj, :],
                            rhs=rband[:, B, :],
                            start=True, stop=True,
                        )
                        if affine:
                            tmp = srng.tile([C, 2, W], BT, tag="tmp", name="tmp")
                            nc.vector.tensor_tensor(
                                out=tmp, in0=xcs[:, 2 * jc : 2 * jc + 2, :],
                                in1=ro[0:C], op=AluOpType.mult,
                            )
                            nc.vector.tensor_scalar(
                                out=lnS[0:C, 2 * jc : 2 * jc + 2, 2:258],
                                in0=tmp, scalar1=gam_s, scalar2=bet_s,
                                op0=AluOpType.mult, op1=AluOpType.add,
                            )
                        else:
                            nc.vector.tensor_tensor(
                                out=lnS[0:C, 2 * jc : 2 * jc + 2, 2:258],
                                in0=xcs[:, 2 * jc : 2 * jc + 2, :],
                                in1=ro[0:C], op=AluOpType.mult,
                            )
                        nc.gpsimd.tensor_copy(
                            lnS[C:128, 2 * jc : 2 * jc + 2, 0:256],
                            lnS[0:C, 2 * jc : 2 * jc + 2, 2:258],
                        )
                        ro_apply = ro  # keep handle: wout reuses other half
                    # ---------------- stage 3: fused w_in + dw3 -> d (lag 18)
                    k = i - BAND - 2
                    if 0 <= k < NCD:
                        prhs = lnS[:, 2 * k : 2 * k + 4, 1:257].rearrange(
                            "p (a b) w -> p a b w", a=2
                        )
                        srhs = lnS[0:C, 2 * k : 2 * k + 4, 2:258].rearrange(
                            "p (a b) w -> p a b w", a=2
                        )
                        for g in range(NG):
                            dp = ps_d.tile([128, 2, W], FT, tag="dp", name="dp")
                            nc.tensor.matmul(
                                dp, lhsT=fdrp_s[:, g], rhs=prhs,
                                start=True, stop=False, perf_mode=DRMODE,
                            )
                            nc.tensor.matmul(
                                dp, lhsT=fdrs_s[:, g], rhs=srhs,
                                start=False, stop=False, perf_mode=DRMODE,
                            )
                            nc.tensor.matmul(
                                dp, lhsT=fnp_s[:, g],
                                rhs=lnS[:, 2 * k + 1 : 2 * k + 3, 1:257],
                                start=False, stop=False,
                            )
                            nc.tensor.matmul(
                                dp, lhsT=fns_s[:, g],
                                rhs=lnS[0:C, 2 * k + 1 : 2 * k + 3, 2:258],
                                start=False, stop=True,
                            )
                            nc.scalar.activation(
                                dts[g][:, 2 * k + 1 : 2 * k + 3, 1:257],
                                dp, mybir.ActivationFunctionType.Copy,
                                scale=float(2.0 ** (SDE - SC1E)),
                            )

                    # ---------------- stage 4 (lag 20)
                    m = i - BAND - 4
                    if 0 <= m < NCO:
                        zts = []
                        for g in range(NG):
                            tp = ps_t.tile([128, 2, W], FT, tag="tp", name="tp")
                            for dx in range(3):
                                drrhs = dts[g][
                                    :, 2 * m + 1 : 2 * m + 5, dx : dx + 256
                                ].rearrange("p (a b) w -> p a b w", a=2)
                                nc.tensor.matmul(
                                    tp, lhsT=wdr_s[:, g, dx], rhs=drrhs,
                                    start=(dx == 0), stop=False, perf_mode=DRMODE,
                                )
                                nc.tensor.matmul(
                                    tp, lhsT=wn_s[:, g, dx],
                                    rhs=dts[g][:, 2 * m + 2 : 2 * m + 4, dx : dx + 256],
                                    start=False, stop=(dx == 2),
                                )
                            th = srng.tile([128, 2, W], BT, tag=f"th{g}", name=f"th{g}")
                            nc.scalar.activation(
                                th, tp, mybir.ActivationFunctionType.Tanh,
                                scale=float(2.0 ** (-SDE - SC2E)),
                            )
                            zt = srng.tile([128, 2, W], BT, tag=f"z{g}", name=f"z{g}")
                            nc.vector.scalar_tensor_tensor(
                                out=zt,
                                in0=dts[g][:, 2 * m + 2 : 2 * m + 4, 1:257],
                                scalar=float(2.0 ** (-SDE)),
                                in1=th, op0=AluOpType.mult, op1=AluOpType.add,
                            )
                            zts.append(zt)
                        g0 = srng.tile([128, 2, W], BT, tag="g0", name="g0")
                        nc.gpsimd.tensor_tensor(
                            out=g0, in0=zts[0], in1=zts[1], op=AluOpType.mult
                        )
                        z2b = srng.tile([42, 2, W], BT, tag="z2b", name="z2b")
                        nc.gpsimd.tensor_copy(z2b, zts[2][64:106])
                        g1 = srng.tile([42, 2, W], BT, tag="g1", name="g1")
                        nc.vector.tensor_tensor(
                            out=g1, in0=zts[2][0:42], in1=z2b, op=AluOpType.mult,
                        )
                        ro2 = ps_ro.tile([128, 2, W], FT, tag="ro", name="ro")
                        nc.tensor.matmul(
                            ro2[C:128], lhsT=wouta_s, rhs=g0, start=True, stop=False
                        )
                        nc.tensor.matmul(
                            ro2[C:128], lhsT=woutb_s, rhs=g1, start=False, stop=True
                        )
                        if m % 4 == 0:
                            ot_cur = ring.tile([C, 8, W], BT, tag="ot", name="ot")
                        nc.vector.tensor_tensor(
                            out=ot_cur[:, m % 4 * 2 : m % 4 * 2 + 2, :],
                            in0=ro2[C:128],
                            in1=x1t[:, 2 * m + 2 : 2 * m + 4, 1:257],
                            op=AluOpType.add,
                        )
                        if m % 4 == 3:
                            nc.sync.dma_start(
                                out=out[
                                    :, 64 * p + 2 * (m - 3) : 64 * p + 2 * (m - 3) + 8, :
                                ],
                                in_=ot_cur,
                            )

    nc.compile()
    return nc


# ---------------------------------------------------------------- host logic

_CACHE = {}


def _programs(affine):
    key = ("k", affine)
    if key not in _CACHE:
        _CACHE[key] = (_build_k1(affine), _build_k2(affine))
    return _CACHE[key]


def _diag_blocks(w, place):
    """w: [340] per-tap vector -> [3,128,128] diag matrices per placed group."""
    out = np.zeros((NG, 128, 128), F32)
    for s, ch in enumerate(place):
        if ch >= 0:
            out[s // 128, s % 128, s % 128] = w[ch]
    return out


def kernel(x, y, ln_w, ln_b, temperature, wq, wq_dw, wkv, wkv_dw, w_proj,
           w_in, w_dw, w_dw1, w_dw2, w_out):
    x = np.asarray(x, F32)
    y = np.asarray(y, F32)
    ln_w = np.asarray(ln_w, F32)
    ln_b = np.asarray(ln_b, F32)
    temperature = np.asarray(temperature, F32)
    wq = np.asarray(wq, F32)
    wq_dw = np.asarray(wq_dw, F32)
    wkv = np.asarray(wkv, F32)
    wkv_dw = np.asarray(wkv_dw, F32)
    w_proj = np.asarray(w_proj, F32)
    w_in = np.asarray(w_in, F32)
    w_dw = np.asarray(w_dw, F32)
    w_dw1 = np.asarray(w_dw1, F32)
    w_dw2 = np.asarray(w_dw2, F32)
    w_out = np.asarray(w_out, F32)

    affine = not (np.allclose(ln_w, 1.0) and np.allclose(ln_b, 0.0))
    k1, k2 = _programs(affine)

    # ---------- launch 1: q/k gram + norms + v (v2)
    xpad = np.zeros((B, C, H + 4, Wp), F32)
    xpad[:, :, 2 : 2 + H, 1 : 1 + W] = x
    ypad = np.zeros((B, C, H + 4, Wp), F32)
    ypad[:, :, 2 : 2 + H, 1 : 1 + W] = y

    SC1 = float(2.0 ** SC1E)

    def _fuse_v2(w1x1, wdw):
        # -> DR pairs [128,2,O], DR singles [64,2,O], norm pair [128,O], norm single [64,O]
        O = w1x1.shape[0]
        drp = np.zeros((128, 2, O), F32)
        drs = np.zeros((C, 2, O), F32)
        npr = np.zeros((128, O), F32)
        nsg = np.zeros((C, O), F32)
        for t, dy in enumerate((0, 2)):
            drp[0:C, t, :] = (w1x1 * wdw[:, 0, dy, 0][:, None]).T
            drp[C:128, t, :] = (w1x1 * wdw[:, 0, dy, 2][:, None]).T
            drs[:, t, :] = (w1x1 * wdw[:, 0, dy, 1][:, None]).T
        npr[0:C, :] = (w1x1 * wdw[:, 0, 1, 0][:, None]).T
        npr[C:128, :] = (w1x1 * wdw[:, 0, 1, 2][:, None]).T
        nsg[:, :] = (w1x1 * wdw[:, 0, 1, 1][:, None]).T
        return drp, drs, npr, nsg

    qdrp, qdrs, qnp_, qns_ = _fuse_v2(wq, wq_dw)
    kdrp, kdrs, knp_, kns_ = _fuse_v2(wkv, wkv_dw)
    vsel = np.zeros((C, BAND, C), F32)
    bsel = np.zeros((BAND, BAND, C), F32)
    for j in range(BAND):
        vsel[:, j, j] = 1.0 / 64.0
        bsel[j, j, :] = 1.0
    common1 = {
        "qdrp": (qdrp * SC1).astype(E4M3),
        "qdrs": (qdrs * SC1).astype(E4M3),
        "qnp": (qnp_ * SC1).astype(E4M3),
        "qns": (qns_ * SC1).astype(E4M3),
        "kdrp": (kdrp * SC1).astype(E4M3),
        "kdrs": (kdrs * SC1).astype(E4M3),
        "knp": (knp_ * SC1).astype(E4M3),
        "kns": (kns_ * SC1).astype(E4M3),
        "vsel": vsel.astype(BF16),
        "bsel": bsel.astype(BF16),
        "identb": np.eye(128).astype(BF16),
    }
    if affine:
        common1["gamv"] = ln_w[:, None].astype(F32)
        common1["betv"] = ln_b[:, None].astype(F32)

    in_maps1 = []
    for core in range(NCORES):
        b, h = core // 2, core % 2
        rs = 1 + h * HS  # padded-coords start row (rows = interior -1..129)
        m = dict(common1)
        m["xh"] = np.ascontiguousarray(xpad[b, :, rs : rs + PH1, :]).astype(BF16)
        m["yh"] = np.ascontiguousarray(ypad[b, :, rs : rs + PH1, :]).astype(BF16)
        in_maps1.append(m)

    res1 = bass_utils.run_bass_kernel_spmd(k1, in_maps1, core_ids=list(range(NCORES)))

    # ---------- host combine: attention softmax -> P = w_proj @ blockdiag(A)
    pts = []
    vfull = np.zeros((B, C, H, W), BF16)
    for b in range(B):
        r0, r1 = res1.results[2 * b], res1.results[2 * b + 1]
        G128 = r0["qg"].astype(np.float64) + r1["qg"].astype(np.float64)
        G = G128[0:C, C:128]
        qss = np.diag(G128[0:C, 0:C])
        kss = np.diag(G128[C:128, C:128])
        nq = np.maximum(np.sqrt(qss), 1e-12)
        nk = np.maximum(np.sqrt(kss), 1e-12)
        A = np.zeros((C, C), np.float64)
        for hh in range(HEADS):
            sl = slice(hh * CH, (hh + 1) * CH)
            logits = temperature[hh, 0, 0] * (G[sl, sl] / np.outer(nq[sl], nk[sl]))
            e = np.exp(logits - logits.max(axis=-1, keepdims=True))
            A[sl, sl] = e / e.sum(axis=-1, keepdims=True)
        P = w_proj.astype(np.float64) @ A
        pts.append(np.ascontiguousarray(P.T).astype(BF16))
        vfull[b, :, 0:HS] = r0["vout"]
        vfull[b, :, HS:H] = r1["vout"]

    # ---------- launch 2: x1 = x + P v ; IEL (v2: fp8 DoubleRow kernel)
    vpad = np.zeros((B, C, H + 4, W), BF16)
    vpad[:, :, 2 : 2 + H, :] = vfull
    xpad16 = xpad.astype(BF16)

    w_in_p = np.zeros((NG * 128, C), F32)
    w_dw_p = np.zeros((NG * 128, 3, 3), F32)
    w12 = np.concatenate([w_dw1[:, 0], w_dw2[:, 0]], axis=0)  # [340,3,3]
    w12_p = np.zeros((NG * 128, 3, 3), F32)
    for s, ch in enumerate(PLACE340):
        if ch >= 0:
            w_in_p[s] = w_in[ch]
            w_dw_p[s] = w_dw[ch, 0]
            w12_p[s] = w12[ch]
    SC1 = float(2.0 ** SC1E)
    SC2 = float(2.0 ** SC2E)
    # fused-d weights: DR pairs (dy=-1,+1), DR singles, norm pair/single (dy=0)
    fdrp = np.zeros((128, NG, 2, 128), F32)
    fdrs = np.zeros((C, NG, 2, 128), F32)
    fnp = np.zeros((128, NG, 128), F32)
    fns = np.zeros((C, NG, 128), F32)
    for g in range(NG):
        sl = slice(g * 128, (g + 1) * 128)
        wi = w_in_p[sl]  # [128m, 64c]
        wd = w_dw_p[sl]  # [128m, 3, 3]
        for t, dy in enumerate((0, 2)):  # tap rows: dy=-1 -> 0, dy=+1 -> 2
            fdrp[0:C, g, t, :] = (wi * wd[:, dy, 0][:, None]).T  # dx=-1 half
            fdrp[C:128, g, t, :] = (wi * wd[:, dy, 2][:, None]).T  # dx=+1 half
            fdrs[:, g, t, :] = (wi * wd[:, dy, 1][:, None]).T
        fnp[0:C, g, :] = (wi * wd[:, 1, 0][:, None]).T
        fnp[C:128, g, :] = (wi * wd[:, 1, 2][:, None]).T
        fns[:, g, :] = (wi * wd[:, 1, 1][:, None]).T
    # dw12 diag weights
    wdr = np.zeros((128, NG, 3, 2, 128), F32)
    wn = np.zeros((128, NG, 3, 128), F32)
    for g in range(NG):
        sl = slice(g * 128, (g + 1) * 128)
        for dx in range(3):
            for t, dy in enumerate((0, 2)):
                wdr[np.arange(128), g, dx, t, np.arange(128)] = w12_p[sl, dy, dx]
            wn[np.arange(128), g, dx, np.arange(128)] = w12_p[sl, 1, dx]
    common2 = {
        "vsel": vsel.astype(BF16),
        "bsel": bsel.astype(BF16),
        "fdrp": (fdrp * SC1).astype(E4M3),
        "fdrs": (fdrs * SC1).astype(E4M3),
        "fnp": (fnp * SC1).astype(E4M3),
        "fns": (fns * SC1).astype(E4M3),
        "wdr": (wdr * SC2).astype(E4M3),
        "wn": (wn * SC2).astype(E4M3),
        "wouta": np.ascontiguousarray(w_out.T[0:128]).astype(BF16),
        "woutb": np.ascontiguousarray(w_out.T[128:170]).astype(BF16),
    }
    if affine:
        common2["gamv"] = ln_w[:, None].astype(F32)
        common2["betv"] = ln_b[:, None].astype(F32)

    in_maps2 = []
    for core in range(NCORES):
        b, h = core // 2, core % 2
        rs = h * HS  # padded-coords start row (halo-2 slab of 132 rows)
        m = dict(common2)
        m["xk"] = np.ascontiguousarray(xpad16[b, :, rs : rs + HS + 4, :])
        m["vk"] = np.ascontiguousarray(vpad[b, :, rs : rs + HS + 4, :])
        m["ptw"] = pts[b]
        in_maps2.append(m)

    res2 = bass_utils.run_bass_kernel_spmd(k2, in_maps2, core_ids=list(range(NCORES)))

    out = np.zeros((B, C, H, W), F32)
    for core in range(NCORES):
        b, h = core // 2, core % 2
        out[b, :, h * HS : (h + 1) * HS, :] = res2.results[core]["out"].astype(F32)
    return out



# revision 11
# speedup vs baseline: 1.7714x; 1.0270x over previous
"""Trainium2 Bass kernel for nn_CSDC_8246337208509 (I_LCA block: CAB cross-attention + IEL gated FFN).

Contract: kernel(**inputs) takes FULL unsharded inputs, returns FULL output.
Sharding: 8 cores = 4 batches x 2 spatial halves (128 rows of H each).
Two device launches with a tiny host-side combine (attention softmax over 8x8
per-head Gram matrices) between them.
"""

import contextlib
import sys

import numpy as np

try:
    import concourse.bass as bass  # noqa: F401
except Exception:  # pragma: no cover
    sys.path.insert(0, "/opt/trn_rl_repo")
    sys.path.insert(0, "/root/.axon_site/_ro/trn_rl_repo")

import concourse.bacc as bacc
import concourse.tile as tile
from concourse import mybir
from concourse import bass_utils
from concourse.alu_op_type import AluOpType
import ml_dtypes

BF16 = ml_dtypes.bfloat16
F32 = np.float32
BT = mybir.dt.bfloat16
FT = mybir.dt.float32

B, C, H, W = 4, 64, 256, 256
HEADS, CH = 8, 8
HID = 170
EPS = 1e-6
Wp = W + 2  # 258, zero col at 0 and 257
HS = H // 2  # 128 interior rows per core
Hb1 = 16  # k1 band interior rows
NB1 = HS // Hb1
Hb2 = 8  # k2 band interior rows
NB2 = HS // Hb2
NCORES = 8

TAPS = [(ty - 1, tx - 1) for ty in range(3) for tx in range(3)]  # (dy, dx), t = ty*3+tx

F8 = mybir.dt.float8e4
E4M3 = ml_dtypes.float8_e4m3
Wf = 272  # fp8 padded row stride (mult of 16)
SC1E = 10  # fused-d weight scale exponent
SDE = 5    # fp8 d storage scale exponent
SC2E = 9   # dw12 weight scale exponent
PH = 68    # k2 pass tile height
NCL = 34
NCD = 33
NCO = 32
BAND = 16
NBAND = 3
DRMODE = mybir.MatmulPerfMode.DoubleRow
PH1 = 130
NCL1 = 65
NCQ = 64
NB1K = 5



# channel placement for the 340-wide IEL stream into 3 groups of 128 partitions:
# G0 = x1[0:128]; G1 = x2[0:128]; G2: slots 0..41 = x1[128:170], slots 64..105 =
# x2[128:170] (partition bases must be 32-aligned, so the x2 tail sits at 64).
PLACE340 = [-1] * 384
for _i in range(128):
    PLACE340[_i] = _i           # G0
    PLACE340[128 + _i] = 170 + _i  # G1
for _i in range(42):
    PLACE340[256 + _i] = 128 + _i  # G2 low: x1 tail
    PLACE340[256 + 64 + _i] = 298 + _i  # G2 high: x2 tail
NG = 3


# ---------------------------------------------------------------- device code

def _ln_into(nc, tc, pools, src, nrows, dst, affine, src_f32=False):
    """Channels-first LayerNorm of src[:, :nrows, 1:257] -> dst (S-stacked bf16).

    src: [64, nrows, 258] bf16 tile. dst: [128, nrows, 260] S-layout tile whose
    pad cols are already zeroed: top half dst[0:64, r, c] = ln[r, c-1] (written
    at cols 2:258), bottom half dst[64:128, r, c] = ln[r, c+1] (gpsimd copy).
    All transposes ride the DMA xbar (bf16), not the PE.
    """
    lnscr = pools["lnscr"]
    T = nrows * 2
    xTs = lnscr.tile([128, T, 64], BT, tag="ln_xTs")
    xnT = lnscr.tile([128, T, 64], BT, tag="ln_xnT")
    st = lnscr.tile([128, T, 6], FT, tag="ln_st")
    mv = lnscr.tile([128, T, 2], FT, tag="ln_mv")
    sr = lnscr.tile([128, T, 1], FT, tag="ln_sr")
    ri = lnscr.tile([128, T, 1], FT, tag="ln_ri")

    ps_t = pools["ps_t"]
    ident = pools["idf"] if src_f32 else pools["idb"]
    for g in range((T + 7) // 8):
        n = min(8, T - g * 8)
        pt = ps_t.tile([128, 8, 64], FT if src_f32 else BT, tag="ps_fw")
        for j in range(n):
            t = g * 8 + j
            row, half = t // 2, t % 2
            nc.tensor.transpose(
                pt[:, j, :],
                src[:, row, 1 + 128 * half : 1 + 128 * half + 128],
                ident[0:64, 0:64],
            )
        (nc.scalar.copy if g % 2 == 0 else nc.vector.tensor_copy)(
            xTs[:, g * 8 : g * 8 + n, :], pt[:, 0:n, :]
        )
    for t in range(T):
        nc.vector.bn_stats(st[:, t, :], xTs[:, t, :])
        nc.vector.bn_aggr(mv[:, t, :], st[:, t, :])
    nc.scalar.activation(sr, mv[:, :, 1:2], mybir.ActivationFunctionType.Sqrt, bias=pools["eps"])
    nc.vector.reciprocal(ri, sr)
    for t in range(T):
        nc.vector.tensor_scalar(
            out=xnT[:, t, :],
            in0=xTs[:, t, :],
            scalar1=mv[:, t, 0:1],
            scalar2=ri[:, t, 0:1],
            op0=AluOpType.subtract,
            op1=AluOpType.mult,
        )
    if affine:
        gam_bc, bet_bc = pools["gam_bc"], pools["bet_bc"]
        for t in range(T):
            nc.vector.tensor_tensor(out=xnT[:, t, :], in0=xnT[:, t, :], in1=gam_bc, op=AluOpType.mult)
            nc.vector.tensor_tensor(out=xnT[:, t, :], in0=xnT[:, t, :], in1=bet_bc, op=AluOpType.add)
    # transpose back (PE) into the S-layout top half, then gpsimd-fill the bottom
    for g in range((T + 3) // 4):
        pb = ps_t.tile([128, 2, 256], BT, tag="ps_bw")
        for j in range(4):
            t = g * 4 + j
            nc.tensor.transpose(
                pb[0:64, j // 2, 128 * (j % 2) : 128 * (j % 2) + 128],
                xnT[:, t, :],
                pools["idb"],
            )
        (nc.scalar.copy if g % 2 == 0 else nc.vector.tensor_copy)(
            dst[0:64, g * 2 : g * 2 + 2, 2:258], pb[0:64]
        )
        nc.gpsimd.tensor_copy(
            dst[64:128, g * 2 : g * 2 + 2, 0:256], dst[0:64, g * 2 : g * 2 + 2, 2:258]
        )


def _zero_pad_cols(nc, t, nrows):
    nc.gpsimd.memset(t[:, 0:nrows, 0:1], 0.0)
    nc.gpsimd.memset(t[:, 0:nrows, 257:258], 0.0)


def _zero_pad_cols_s(nc, t, nrows):
    # S-stacked layout [128, nrows, 260]: top half holds u[c-1], bottom u[c+1]
    nc.gpsimd.memset(t[0:64, 0:nrows, 0:2], 0.0)
    nc.gpsimd.memset(t[0:64, 0:nrows, 258:260], 0.0)
    nc.gpsimd.memset(t[64:128, 0:nrows, 256:260], 0.0)


# fused conv1x1+dw3x3: 3 K=128 pair-matmuls + 3 K=64 single-matmuls per chunk.
# S: [128, nr, 260] stacked input; out rows j correspond to S rows j+roff.
def _fused_conv(nc, ps_pool, pairs, sings, S, roff, nchunks, evict, M=128):
    for c in range(nchunks):
        pt = ps_pool.tile([128, 2, W], FT, tag="ps_mm")
        for p in range(3):
            dy = p - 1
            nc.tensor.matmul(
                pt[0:M],
                lhsT=pairs[:, p, :],
                rhs=S[:, roff + 2 * c + dy : roff + 2 * c + dy + 2, 1:257],
                start=(p == 0),
                stop=False,
            )
        for i in range(3):
            dy = i - 1
            nc.tensor.matmul(
                pt[0:M],
                lhsT=sings[:, i, :],
                rhs=S[0:64, roff + 2 * c + dy : roff + 2 * c + dy + 2, 2:258],
                start=False,
                stop=(i == 2),
            )
        evict(c, pt[0:M])


def _build_k1(affine):
    nc = bacc.Bacc("TRN2", target_bir_lowering=False, debug=False)
    xh = nc.dram_tensor("xh", [C, PH1, Wp], BT, kind="ExternalInput").ap()
    yh = nc.dram_tensor("yh", [C, PH1, Wp], BT, kind="ExternalInput").ap()
    qdrp = nc.dram_tensor("qdrp", [128, 2, C], F8, kind="ExternalInput").ap()
    qdrs = nc.dram_tensor("qdrs", [C, 2, C], F8, kind="ExternalInput").ap()
    qnp = nc.dram_tensor("qnp", [128, C], F8, kind="ExternalInput").ap()
    qns = nc.dram_tensor("qns", [C, C], F8, kind="ExternalInput").ap()
    kdrp = nc.dram_tensor("kdrp", [128, 2, 2 * C], F8, kind="ExternalInput").ap()
    kdrs = nc.dram_tensor("kdrs", [C, 2, 2 * C], F8, kind="ExternalInput").ap()
    knp = nc.dram_tensor("knp", [128, 2 * C], F8, kind="ExternalInput").ap()
    kns = nc.dram_tensor("kns", [C, 2 * C], F8, kind="ExternalInput").ap()
    vsel = nc.dram_tensor("vsel", [C, BAND, C], BT, kind="ExternalInput").ap()
    bsel = nc.dram_tensor("bsel", [BAND, BAND, C], BT, kind="ExternalInput").ap()
    identb = nc.dram_tensor("identb", [128, 128], BT, kind="ExternalInput").ap()
    if affine:
        gamv = nc.dram_tensor("gamv", [C, 1], FT, kind="ExternalInput").ap()
        betv = nc.dram_tensor("betv", [C, 1], FT, kind="ExternalInput").ap()

    qg = nc.dram_tensor("qg", [128, 128], FT, kind="ExternalOutput").ap()
    vout = nc.dram_tensor("vout", [C, 2 * C, W], BT, kind="ExternalOutput").ap()

    with tile.TileContext(nc) as tc:
        with contextlib.ExitStack() as ctx:
            wp = ctx.enter_context(tc.tile_pool(name="wp", bufs=1))
            big = ctx.enter_context(tc.tile_pool(name="big", bufs=1))
            ring = ctx.enter_context(tc.tile_pool(name="ring", bufs=3))
            srng = ctx.enter_context(tc.tile_pool(name="srng", bufs=2))
            ps_ub = ctx.enter_context(tc.tile_pool(name="ps_ub", bufs=2, space="PSUM"))
            ps_v = ctx.enter_context(tc.tile_pool(name="ps_v", bufs=1, space="PSUM"))
            ps_r = ctx.enter_context(tc.tile_pool(name="ps_r", bufs=1, space="PSUM"))
            ps_cv = ctx.enter_context(tc.tile_pool(name="ps_cv", bufs=2, space="PSUM"))
            ps_tp = ctx.enter_context(tc.tile_pool(name="ps_tp", bufs=1, space="PSUM"))
            ps_g = ctx.enter_context(tc.tile_pool(name="ps_g", bufs=1, space="PSUM"))

            qdrp_s = wp.tile([128, 2, C], F8)
            nc.sync.dma_start(out=qdrp_s, in_=qdrp)
            qdrs_s = wp.tile([C, 2, C], F8)
            nc.sync.dma_start(out=qdrs_s, in_=qdrs)
            qnp_s = wp.tile([128, C], F8)
            nc.sync.dma_start(out=qnp_s, in_=qnp)
            qns_s = wp.tile([C, C], F8)
            nc.sync.dma_start(out=qns_s, in_=qns)
            kdrp_s = wp.tile([128, 2, 2 * C], F8)
            nc.sync.dma_start(out=kdrp_s, in_=kdrp)
            kdrs_s = wp.tile([C, 2, 2 * C], F8)
            nc.sync.dma_start(out=kdrs_s, in_=kdrs)
            knp_s = wp.tile([128, 2 * C], F8)
            nc.sync.dma_start(out=knp_s, in_=knp)
            kns_s = wp.tile([C, 2 * C], F8)
            nc.sync.dma_start(out=kns_s, in_=kns)
            vsel_s = wp.tile([C, BAND, C], BT)
            nc.sync.dma_start(out=vsel_s, in_=vsel)
            bsel_s = wp.tile([BAND, BAND, C], BT)
            nc.sync.dma_start(out=bsel_s, in_=bsel)
            id_s = wp.tile([128, 128], BT)
            nc.sync.dma_start(out=id_s, in_=identb)
            usel_s = wp.tile([C, C], BT)
            nc.vector.memset(usel_s, 1.0 / 64.0)
            eps_s = wp.tile([128, 1], FT)
            nc.vector.memset(eps_s, EPS)
            if affine:
                gam_s = wp.tile([C, 1], FT)
                nc.sync.dma_start(out=gam_s, in_=gamv)
                bet_s = wp.tile([C, 1], FT)
                nc.sync.dma_start(out=bet_s, in_=betv)

            lnSx = big.tile([128, PH1, Wf], F8, name="lnSx")
            lnSy = big.tile([128, PH1, Wf], F8, name="lnSy")
            xy = big.tile([128, PH1, W], BT, name="xy")  # top xc, bottom yc
            rbx = big.tile([BAND, NB1K, 512], BT, name="rbx")
            rby = big.tile([BAND, NB1K, 512], BT, name="rby")

            for S in (lnSx, lnSy):
                nc.gpsimd.memset(S[0:C, :, 0:2], 0.0)
                nc.gpsimd.memset(S[0:C, :, 258:Wf], 0.0)
                nc.gpsimd.memset(S[C:128, :, 256:Wf], 0.0)

            gp = ps_g.tile([128, 128], FT, name="gp")

            nrow_grp = [8] * 16 + [2]
            xbs = ybs = None
            vps_cur = None
            vt_cur = None

            for i in range(NCQ + 22):
                # ---------------- stage 1: moments
                c = i
                if c < NCL1:
                    if c % 4 == 0:
                        g4 = c // 4
                        nr = nrow_grp[g4]
                        xb = ring.tile([C, 8, Wp], BT, tag="xb", name="xb")
                        nc.sync.dma_start(
                            out=xb[:, 0:nr, :], in_=xh[:, 8 * g4 : 8 * g4 + nr, :]
                        )
                        yb = ring.tile([C, 8, Wp], BT, tag="yb", name="yb")
                        nc.sync.dma_start(
                            out=yb[:, 0:nr, :], in_=yh[:, 8 * g4 : 8 * g4 + nr, :]
                        )
                        xbs, ybs = xb, yb
                    lr = c % 4 * 2
                    ubt = ps_ub.tile([128, 2, W], FT, tag="ubt", name="ubt")
                    nc.tensor.matmul(
                        ubt[0:C], lhsT=usel_s, rhs=xbs[:, lr : lr + 2, 1:257],
                        start=True, stop=True,
                    )
                    nc.tensor.matmul(
                        ubt[C:128], lhsT=usel_s, rhs=ybs[:, lr : lr + 2, 1:257],
                        start=True, stop=True,
                    )
                    nc.vector.tensor_tensor(
                        out=xy[0:C, 2 * c : 2 * c + 2, :],
                        in0=xbs[:, lr : lr + 2, 1:257], in1=ubt[0:C],
                        op=AluOpType.subtract,
                    )
                    nc.vector.tensor_tensor(
                        out=xy[C:128, 2 * c : 2 * c + 2, :],
                        in0=ybs[:, lr : lr + 2, 1:257], in1=ubt[C:128],
                        op=AluOpType.subtract,
                    )
                    xq = srng.tile([C, 2, W], BT, tag="xq", name="xq")
                    nc.vector.tensor_tensor(
                        out=xq, in0=xy[0:C, 2 * c : 2 * c + 2, :],
                        in1=xy[0:C, 2 * c : 2 * c + 2, :], op=AluOpType.mult,
                    )
                    yq = srng.tile([C, 2, W], BT, tag="yq", name="yq")
                    nc.vector.tensor_tensor(
                        out=yq, in0=xy[C:128, 2 * c : 2 * c + 2, :],
                        in1=xy[C:128, 2 * c : 2 * c + 2, :], op=AluOpType.mult,
                    )
                    B = c // BAND
                    j = c % BAND
                    if j == 0:
                        vps_cur = ps_v.tile([128, 512], FT, tag="vps", name="vps")
                    last = j == BAND - 1 or c == NCL1 - 1
                    nc.tensor.matmul(
                        vps_cur[0:C], lhsT=vsel_s[:, j, :], rhs=xq,
                        start=(j == 0), stop=last,
                    )
                    nc.tensor.matmul(
                        vps_cur[C:128], lhsT=vsel_s[:, j, :], rhs=yq,
                        start=(j == 0), stop=last,
                    )
                    if last:
                        sbx = srng.tile([BAND, 512], BT, tag="sbx", name="sbx")
                        nc.scalar.activation(
                            sbx, vps_cur[0:BAND],
                            mybir.ActivationFunctionType.Sqrt, bias=eps_s[0:BAND],
                        )
                        sby = srng.tile([BAND, 512], BT, tag="sby", name="sby")
                        nc.scalar.activation(
                            sby, vps_cur[C : C + BAND],
                            mybir.ActivationFunctionType.Sqrt, bias=eps_s[0:BAND],
                        )
                        with nc.allow_low_precision(reason="bf16 rsqrt rows"):
                            nc.vector.reciprocal(rbx[:, B, :], sbx)
                            nc.vector.reciprocal(rby[:, B, :], sby)

                # ---------------- stage 2: apply (lag 16)
                jc = i - BAND
                if 0 <= jc < NCL1:
                    B = jc // BAND
                    jj = jc % BAND
                    rbc = ps_r.tile([128, 2, W], FT, tag="rbc", name="rbc")
                    nc.tensor.matmul(
                        rbc[0:C], lhsT=bsel_s[:, jj, :], rhs=rbx[:, B, :],
                        start=True, stop=True,
                    )
                    nc.tensor.matmul(
                        rbc[C:128], lhsT=bsel_s[:, jj, :], rhs=rby[:, B, :],
                        start=True, stop=True,
                    )
                    rbs = srng.tile([128, 2, W], BT, tag="rbs", name="rbs", bufs=3)
                    nc.scalar.copy(rbs, rbc)
                    for S, half in ((lnSx, 0), (lnSy, 1)):
                        src = xy[half * C : half * C + C, 2 * jc : 2 * jc + 2, :]
                        if affine:
                            tmp = srng.tile([C, 2, W], BT, tag=f"tmp{half}", name=f"tmp{half}")
                            nc.vector.tensor_tensor(
                                out=tmp, in0=src,
                                in1=rbs[half * C : half * C + C],
                                op=AluOpType.mult,
                            )
                            nc.vector.tensor_scalar(
                                out=S[0:C, 2 * jc : 2 * jc + 2, 2:258],
                                in0=tmp, scalar1=gam_s, scalar2=bet_s,
                                op0=AluOpType.mult, op1=AluOpType.add,
                            )
                        else:
                            nc.vector.tensor_tensor(
                                out=S[0:C, 2 * jc : 2 * jc + 2, 2:258],
                                in0=src, in1=rbs[half * C : half * C + C],
                                op=AluOpType.mult,
                            )
                        nc.gpsimd.tensor_copy(
                            S[C:128, 2 * jc : 2 * jc + 2, 0:256],
                            S[0:C, 2 * jc : 2 * jc + 2, 2:258],
                        )

                # ---------------- stage 3: q/kv conv, evicts, transposes (lag 18)
                m = i - BAND - 2
                if 0 <= m < NCQ:
                    qk = srng.tile([128, 2, W], BT, tag="qk", name="qk", bufs=3)
                    for S, drp, drs, np_, ns_, MM in (
                        (lnSx, qdrp_s, qdrs_s, qnp_s, qns_s, C),
                        (lnSy, kdrp_s, kdrs_s, knp_s, kns_s, 2 * C),
                    ):
                        cv = ps_cv.tile([128, 2, W], FT, tag="cv", name="cv")
                        prhs = S[:, 2 * m : 2 * m + 4, 1:257].rearrange(
                            "p (a b) w -> p a b w", a=2
                        )
                        srhs = S[0:C, 2 * m : 2 * m + 4, 2:258].rearrange(
                            "p (a b) w -> p a b w", a=2
                        )
                        nc.tensor.matmul(
                            cv[0:MM], lhsT=drp, rhs=prhs,
                            start=True, stop=False, perf_mode=DRMODE,
                        )
                        nc.tensor.matmul(
                            cv[0:MM], lhsT=drs, rhs=srhs,
                            start=False, stop=False, perf_mode=DRMODE,
                        )
                        nc.tensor.matmul(
                            cv[0:MM], lhsT=np_,
                            rhs=S[:, 2 * m + 1 : 2 * m + 3, 1:257],
                            start=False, stop=False,
                        )
                        nc.tensor.matmul(
                            cv[0:MM], lhsT=ns_,
                            rhs=S[0:C, 2 * m + 1 : 2 * m + 3, 2:258],
                            start=False, stop=True,
                        )
                        if MM == C:
                            nc.scalar.activation(
                                qk[0:C], cv[0:C],
                                mybir.ActivationFunctionType.Copy,
                                scale=float(2.0 ** (-SC1E)),
                            )
                        else:
                            nc.scalar.activation(
                                qk[C:128], cv[0:C],
                                mybir.ActivationFunctionType.Copy,
                                scale=float(2.0 ** (-SC1E)),
                            )
                            if m % 4 == 0:
                                vt_cur = ring.tile([C, 8, W], BT, tag="vt", name="vt")
                            nc.scalar.activation(
                                vt_cur[:, m % 4 * 2 : m % 4 * 2 + 2, :],
                                cv[C : 2 * C],
                                mybir.ActivationFunctionType.Copy,
                                scale=float(2.0 ** (-SC1E)),
                            )
                            if m % 4 == 3:
                                nc.sync.dma_start(
                                    out=vout[:, 2 * (m - 3) : 2 * (m - 3) + 8, :],
                                    in_=vt_cur,
                                )
                    tp = ps_tp.tile([128, 4, 128], BT, tag="tp", name="tp")
                    for b in range(4):
                        nc.tensor.transpose(
                            tp[:, b, :], qk[:, b // 2, 128 * (b % 2) : 128 * (b % 2) + 128],
                            id_s,
                        )
                    qkT = srng.tile([128, 4, 128], BT, tag="qkT", name="qkT", bufs=3)
                    nc.scalar.copy(qkT, tp)
                    for b in range(4):
                        nc.tensor.matmul(
                            gp, lhsT=qkT[:, b, :], rhs=qkT[:, b, :],
                            start=(m == 0 and b == 0), stop=(m == NCQ - 1 and b == 3),
                        )

            gsb = wp.tile([128, 128], FT)
            nc.vector.tensor_copy(gsb, gp)
            nc.sync.dma_start(out=qg, in_=gsb)

    nc.compile()
    return nc


def _build_k2(affine):
    nc = bacc.Bacc("TRN2", target_bir_lowering=False, debug=False)
    xk = nc.dram_tensor("xk", [C, 132, Wp], BT, kind="ExternalInput").ap()
    vk = nc.dram_tensor("vk", [C, 132, W], BT, kind="ExternalInput").ap()
    ptw = nc.dram_tensor("ptw", [C, C], BT, kind="ExternalInput").ap()
    vsel = nc.dram_tensor("vsel", [C, BAND, C], BT, kind="ExternalInput").ap()
    bsel = nc.dram_tensor("bsel", [BAND, BAND, C], BT, kind="ExternalInput").ap()
    fdrp = nc.dram_tensor("fdrp", [128, NG, 2, 128], F8, kind="ExternalInput").ap()
    fdrs = nc.dram_tensor("fdrs", [C, NG, 2, 128], F8, kind="ExternalInput").ap()
    fnp = nc.dram_tensor("fnp", [128, NG, 128], F8, kind="ExternalInput").ap()
    fns = nc.dram_tensor("fns", [C, NG, 128], F8, kind="ExternalInput").ap()
    wdr = nc.dram_tensor("wdr", [128, NG, 3, 2, 128], F8, kind="ExternalInput").ap()
    wn = nc.dram_tensor("wn", [128, NG, 3, 128], F8, kind="ExternalInput").ap()
    wouta = nc.dram_tensor("wouta", [128, C], BT, kind="ExternalInput").ap()
    woutb = nc.dram_tensor("woutb", [42, C], BT, kind="ExternalInput").ap()
    if affine:
        gamv = nc.dram_tensor("gamv", [C, 1], FT, kind="ExternalInput").ap()
        betv = nc.dram_tensor("betv", [C, 1], FT, kind="ExternalInput").ap()

    out = nc.dram_tensor("out", [C, HS, W], BT, kind="ExternalOutput").ap()

    with tile.TileContext(nc) as tc:
        with contextlib.ExitStack() as ctx:
            wp = ctx.enter_context(tc.tile_pool(name="wp", bufs=1))
            big = ctx.enter_context(tc.tile_pool(name="big", bufs=1))
            ring = ctx.enter_context(tc.tile_pool(name="ring", bufs=3))
            srng = ctx.enter_context(tc.tile_pool(name="srng", bufs=2))
            ps_pu = ctx.enter_context(tc.tile_pool(name="ps_pu", bufs=2, space="PSUM"))
            ps_ro = ctx.enter_context(tc.tile_pool(name="ps_ro", bufs=2, space="PSUM"))
            ps_var = ctx.enter_context(tc.tile_pool(name="ps_var", bufs=1, space="PSUM"))
            ps_d = ctx.enter_context(tc.tile_pool(name="ps_d", bufs=2, space="PSUM"))
            ps_t = ctx.enter_context(tc.tile_pool(name="ps_t", bufs=1, space="PSUM"))

            # ------------------------------------------------ persistent weights
            ptw_s = wp.tile([C, C], BT)
            nc.sync.dma_start(out=ptw_s, in_=ptw)
            vsel_s = wp.tile([C, BAND, C], BT)
            nc.sync.dma_start(out=vsel_s, in_=vsel)
            bsel_s = wp.tile([BAND, BAND, C], BT)
            nc.sync.dma_start(out=bsel_s, in_=bsel)
            fdrp_s = wp.tile([128, NG, 2, 128], F8)
            nc.sync.dma_start(out=fdrp_s, in_=fdrp)
            fdrs_s = wp.tile([C, NG, 2, 128], F8)
            nc.sync.dma_start(out=fdrs_s, in_=fdrs)
            fnp_s = wp.tile([128, NG, 128], F8)
            nc.sync.dma_start(out=fnp_s, in_=fnp)
            fns_s = wp.tile([C, NG, 128], F8)
            nc.sync.dma_start(out=fns_s, in_=fns)
            wdr_s = wp.tile([128, NG, 3, 2, 128], F8)
            nc.sync.dma_start(out=wdr_s, in_=wdr)
            wn_s = wp.tile([128, NG, 3, 128], F8)
            nc.sync.dma_start(out=wn_s, in_=wn)
            wouta_s = wp.tile([128, C], BT)
            nc.sync.dma_start(out=wouta_s, in_=wouta)
            woutb_s = wp.tile([42, C], BT)
            nc.sync.dma_start(out=woutb_s, in_=woutb)
            usel_s = wp.tile([C, C], BT)
            nc.vector.memset(usel_s, 1.0 / 64.0)
            ones1 = wp.tile([1, C], BT)
            nc.vector.memset(ones1, 1.0)
            eps_s = wp.tile([128, 1], FT)
            nc.vector.memset(eps_s, EPS)
            if affine:
                gam_s = wp.tile([C, 1], FT)
                nc.sync.dma_start(out=gam_s, in_=gamv)
                bet_s = wp.tile([C, 1], FT)
                nc.sync.dma_start(out=bet_s, in_=betv)

            for p in range(2):
                d0 = 64 * p  # dram row of local row 0

                xt = big.tile([128, PH, Wp], BT, tag="x1", name=f"x1_{p}")
                x1t = xt[0:C]
                xcs = xt[C:128, :, 1:257]
                lnS = big.tile([128, PH, Wf], F8, tag="lnS", name=f"lnS_{p}")
                dts = [
                    big.tile([128, PH, Wf], F8, tag=f"d{g}", name=f"d{g}_{p}")
                    for g in range(NG)
                ]
                rband = big.tile([BAND, NBAND, 512], BT, tag="rband", name=f"rband_{p}")

                nc.gpsimd.memset(lnS[0:C, :, 0:2], 0.0)
                nc.gpsimd.memset(lnS[0:C, :, 258:Wf], 0.0)
                nc.gpsimd.memset(lnS[C:128, :, 256:Wf], 0.0)
                for g in range(NG):
                    nc.gpsimd.memset(dts[g][:, :, 0:1], 0.0)
                    nc.gpsimd.memset(dts[g][:, :, 257:Wf], 0.0)

                nrow_grp = [8] * 8 + [4]
                xbs = vbs = None
                vps_cur = None
                ot_cur = None

                for i in range(NCO + 20):
                    # ---------------- stage 1
                    c = i
                    if c < NCL:
                        if c % 4 == 0:
                            g4 = c // 4
                            nr = nrow_grp[g4]
                            xb = ring.tile([C, 8, Wp], BT, tag="xb", name="xb")
                            nc.sync.dma_start(
                                out=xb[:, 0:nr, :],
                                in_=xk[:, d0 + 8 * g4 : d0 + 8 * g4 + nr, :],
                            )
                            vb = ring.tile([C, 8, W], BT, tag="vb", name="vb")
                            nc.sync.dma_start(
                                out=vb[:, 0:nr, :],
                                in_=vk[:, d0 + 8 * g4 : d0 + 8 * g4 + nr, :],
                            )
                            xbs, vbs = xb, vb
                        lr = c % 4 * 2
                        pu = ps_pu.tile([128, 2, W], FT, tag="pu", name="pu")
                        nc.tensor.matmul(
                            pu[0:C], lhsT=ptw_s, rhs=vbs[:, lr : lr + 2, :],
                            start=True, stop=True,
                        )
                        nc.vector.tensor_tensor(
                            out=x1t[:, 2 * c : 2 * c + 2, 1:257],
                            in0=pu[0:C],
                            in1=xbs[:, lr : lr + 2, 1:257],
                            op=AluOpType.add,
                        )
                        nc.tensor.matmul(
                            pu[C:128], lhsT=usel_s,
                            rhs=x1t[:, 2 * c : 2 * c + 2, 1:257],
                            start=True, stop=True,
                        )
                        nc.vector.tensor_tensor(
                            out=xcs[:, 2 * c : 2 * c + 2, :],
                            in0=x1t[:, 2 * c : 2 * c + 2, 1:257],
                            in1=pu[C:128],
                            op=AluOpType.subtract,
                        )
                        xq = srng.tile([C, 2, W], BT, tag="xq", name="xq")
                        nc.vector.tensor_tensor(
                            out=xq, in0=xcs[:, 2 * c : 2 * c + 2, :],
                            in1=xcs[:, 2 * c : 2 * c + 2, :], op=AluOpType.mult,
                        )
                        B = c // BAND
                        j = c % BAND
                        if j == 0:
                            vps_cur = ps_var.tile([C, 512], FT, tag="vps", name="vps")
                        nc.tensor.matmul(
                            vps_cur, lhsT=vsel_s[:, j, :], rhs=xq,
                            start=(j == 0), stop=(j == BAND - 1 or c == NCL - 1),
                        )
                        if j == BAND - 1 or c == NCL - 1:
                            sb = srng.tile([BAND, 512], BT, tag="sb", name="sb")
                            nc.scalar.activation(
                                sb, vps_cur[0:BAND], mybir.ActivationFunctionType.Sqrt,
                                bias=eps_s[0:BAND],
                            )
                            with nc.allow_low_precision(reason="bf16 rsqrt rows"):
                                nc.vector.reciprocal(rband[:, B, :], sb)

                    # ---------------- stage 2: apply (lag 16)
                    jc = i - BAND
                    if 0 <= jc < NCL:
                        B = jc // BAND
                        jj = jc % BAND
                        ro = ps_ro.tile([128, 2, W], FT, tag="ro", name="ro")
                        nc.tensor.matmul(
                            ro[0:C], lhsT=bsel_s[:, jj, :],
                            rhs=rband[:, B, :],
                            start=True, stop=True,
                        )
                        if affine:
                            tmp = srng.tile([C, 2, W], BT, tag="tmp", name="tmp")
                            nc.vector.tensor_tensor(
                                out=tmp, in0=xcs[:, 2 * jc : 2 * jc + 2, :],
                                in1=ro[0:C], op=AluOpType.mult,
                            )
                            nc.vector.tensor_scalar(
                                out=lnS[0:C, 2 * jc : 2 * jc + 2, 2:258],
                                in0=tmp, scalar1=gam_s, scalar2=bet_s,
                                op0=AluOpType.mult, op1=AluOpType.add,
                            )
                        else:
                            nc.vector.tensor_tensor(
                                out=lnS[0:C, 2 * jc : 2 * jc + 2, 2:258],
                                in0=xcs[:, 2 * jc : 2 * jc + 2, :],
                                in1=ro[0:C], op=AluOpType.mult,
                            )
                        nc.gpsimd.tensor_copy(
                            lnS[C:128, 2 * jc : 2 * jc + 2, 0:256],
                            lnS[0:C, 2 * jc : 2 * jc + 2, 2:258],
                        )
                        ro_apply = ro  # keep handle: wout reuses other half
                    # ---------------- stage 3: fused w_in + dw3 -> d (lag 18)
                    k = i - BAND - 2
                    if 0 <= k < NCD:
                        prhs = lnS[:, 2 * k : 2 * k + 4, 1:257].rearrange(
                            "p (a b) w -> p a b w", a=2
                        )
                        srhs = lnS[0:C, 2 * k : 2 * k + 4, 2:258].rearrange(
                            "p (a b) w -> p a b w", a=2
                        )
                        for g in range(NG):
                            dp = ps_d.tile([128, 2, W], FT, tag="dp", name="dp")
                            nc.tensor.matmul(
                                dp, lhsT=fdrp_s[:, g], rhs=prhs,
                                start=True, stop=False, perf_mode=DRMODE,
                            )
                            nc.tensor.matmul(
                                dp, lhsT=fdrs_s[:, g], rhs=srhs,
                                start=False, stop=False, perf_mode=DRMODE,
                            )
                            nc.tensor.matmul(
                                dp, lhsT=fnp_s[:, g],
                                rhs=lnS[:, 2 * k + 1 : 2 * k + 3, 1:257],
                                start=False, stop=False,
                            )
                            nc.tensor.matmul(
                                dp, lhsT=fns_s[:, g],
                                rhs=lnS[0:C, 2 * k + 1 : 2 * k + 3, 2:258],
                                start=False, stop=True,
                            )
                            nc.scalar.activation(
                                dts[g][:, 2 * k + 1 : 2 * k + 3, 1:257],
                                dp, mybir.ActivationFunctionType.Copy,
                                scale=float(2.0 ** (SDE - SC1E)),
                            )

                    # ---------------- stage 4 (lag 20)
                    m = i - BAND - 4
                    if 0 <= m < NCO:
                        zts = []
                        for g in range(NG):
                            tp = ps_t.tile([128, 2, W], FT, tag="tp", name="tp")
                            for dx in range(3):
                                drrhs = dts[g][
                                    :, 2 * m + 1 : 2 * m + 5, dx : dx + 256
                                ].rearrange("p (a b) w -> p a b w", a=2)
                                nc.tensor.matmul(
                                    tp, lhsT=wdr_s[:, g, dx], rhs=drrhs,
                                    start=(dx == 0), stop=False, perf_mode=DRMODE,
                                )
                                nc.tensor.matmul(
                                    tp, lhsT=wn_s[:, g, dx],
                                    rhs=dts[g][:, 2 * m + 2 : 2 * m + 4, dx : dx + 256],
                                    start=False, stop=(dx == 2),
                                )
                            th = srng.tile([128, 2, W], BT, tag=f"th{g}", name=f"th{g}")
                            nc.scalar.activation(
                                th, tp, mybir.ActivationFunctionType.Tanh,
                                scale=float(2.0 ** (-SDE - SC2E)),
                            )
                            zt = srng.tile([128, 2, W], BT, tag=f"z{g}", name=f"z{g}")
                            nc.vector.scalar_tensor_tensor(
                                out=zt,
                                in0=dts[g][:, 2 * m + 2 : 2 * m + 4, 1:257],
                                scalar=float(2.0 ** (-SDE)),
                                in1=th, op0=AluOpType.mult, op1=AluOpType.add,
                            )
                            zts.append(zt)
                        g0 = srng.tile([128, 2, W], BT, tag="g0", name="g0")
                        nc.gpsimd.tensor_tensor(
                            out=g0, in0=zts[0], in1=zts[1], op=AluOpType.mult
                        )
                        z2b = srng.tile([42, 2, W], BT, tag="z2b", name="z2b")
                        nc.gpsimd.tensor_copy(z2b, zts[2][64:106])
                        g1 = srng.tile([42, 2, W], BT, tag="g1", name="g1")
                        nc.vector.tensor_tensor(
                            out=g1, in0=zts[2][0:42], in1=z2b, op=AluOpType.mult,
                        )
                        ro2 = ps_ro.tile([128, 2, W], FT, tag="ro", name="ro")
                        nc.tensor.matmul(
                            ro2[C:128], lhsT=wouta_s, rhs=g0, start=True, stop=False
                        )
                        nc.tensor.matmul(
                            ro2[C:128], lhsT=woutb_s, rhs=g1, start=False, stop=True
                        )
                        if m % 4 == 0:
                            ot_cur = ring.tile([C, 8, W], BT, tag="ot", name="ot")
                        nc.vector.tensor_tensor(
                            out=ot_cur[:, m % 4 * 2 : m % 4 * 2 + 2, :],
                            in0=ro2[C:128],
                            in1=x1t[:, 2 * m + 2 : 2 * m + 4, 1:257],
                            op=AluOpType.add,
                        )
                        if m % 4 == 3:
                            nc.sync.dma_start(
                                out=out[
                                    :, 64 * p + 2 * (m - 3) : 64 * p + 2 * (m - 3) + 8, :
                                ],
                                in_=ot_cur,
                            )

    nc.compile()
    return nc


# ---------------------------------------------------------------- host logic

_CACHE = {}


def _programs(affine):
    key = ("k", affine)
    if key not in _CACHE:
        _CACHE[key] = (_build_k1(affine), _build_k2(affine))
    return _CACHE[key]


def _diag_blocks(w, place):
    """w: [340] per-tap vector -> [3,128,128] diag matrices per placed group."""
    out = np.zeros((NG, 128, 128), F32)
    for s, ch in enumerate(place):
        if ch >= 0:
            out[s // 128, s % 128, s % 128] = w[ch]
    return out


def kernel(x, y, ln_w, ln_b, temperature, wq, wq_dw, wkv, wkv_dw, w_proj,
           w_in, w_dw, w_dw1, w_dw2, w_out):
    x = np.asarray(x, F32)
    y = np.asarray(y, F32)
    ln_w = np.asarray(ln_w, F32)
    ln_b = np.asarray(ln_b, F32)
    temperature = np.asarray(temperature, F32)
    wq = np.asarray(wq, F32)
    wq_dw = np.asarray(wq_dw, F32)
    wkv = np.asarray(wkv, F32)
    wkv_dw = np.asarray(wkv_dw, F32)
    w_proj = np.asarray(w_proj, F32)
    w_in = np.asarray(w_in, F32)
    w_dw = np.asarray(w_dw, F32)
    w_dw1 = np.asarray(w_dw1, F32)
    w_dw2 = np.asarray(w_dw2, F32)
    w_out = np.asarray(w_out, F32)

    affine = not (np.allclose(ln_w, 1.0) and np.allclose(ln_b, 0.0))
    k1, k2 = _programs(affine)

    # ---------- launch 1: q/k gram + norms + v (v2)
    xpad = np.zeros((B, C, H + 4, Wp), F32)
    xpad[:, :, 2 : 2 + H, 1 : 1 + W] = x
    ypad = np.zeros((B, C, H + 4, Wp), F32)
    ypad[:, :, 2 : 2 + H, 1 : 1 + W] = y

    SC1 = float(2.0 ** SC1E)

    def _fuse_v2(w1x1, wdw):
        # -> DR pairs [128,2,O], DR singles [64,2,O], norm pair [128,O], norm single [64,O]
        O = w1x1.shape[0]
        drp = np.zeros((128, 2, O), F32)
        drs = np.zeros((C, 2, O), F32)
        npr = np.zeros((128, O), F32)
        nsg = np.zeros((C, O), F32)
        for t, dy in enumerate((0, 2)):
            drp[0:C, t, :] = (w1x1 * wdw[:, 0, dy, 0][:, None]).T
            drp[C:128, t, :] = (w1x1 * wdw[:, 0, dy, 2][:, None]).T
            drs[:, t, :] = (w1x1 * wdw[:, 0, dy, 1][:, None]).T
        npr[0:C, :] = (w1x1 * wdw[:, 0, 1, 0][:, None]).T
        npr[C:128, :] = (w1x1 * wdw[:, 0, 1, 2][:, None]).T
        nsg[:, :] = (w1x1 * wdw[:, 0, 1, 1][:, None]).T
        return drp, drs, npr, nsg

    qdrp, qdrs, qnp_, qns_ = _fuse_v2(wq, wq_dw)
    kdrp, kdrs, knp_, kns_ = _fuse_v2(wkv, wkv_dw)
    vsel = np.zeros((C, BAND, C), F32)
    bsel = np.zeros((BAND, BAND, C), F32)
    for j in range(BAND):
        vsel[:, j, j] = 1.0 / 64.0
        bsel[j, j, :] = 1.0
    common1 = {
        "qdrp": (qdrp * SC1).astype(E4M3),
        "qdrs": (qdrs * SC1).astype(E4M3),
        "qnp": (qnp_ * SC1).astype(E4M3),
        "qns": (qns_ * SC1).astype(E4M3),
        "kdrp": (kdrp * SC1).astype(E4M3),
        "kdrs": (kdrs * SC1).astype(E4M3),
        "knp": (knp_ * SC1).astype(E4M3),
        "kns": (kns_ * SC1).astype(E4M3),
        "vsel": vsel.astype(BF16),
        "bsel": bsel.astype(BF16),
        "identb": np.eye(128).astype(BF16),
    }
    if affine:
        common1["gamv"] = ln_w[:, None].astype(F32)
        common1["betv"] = ln_b[:, None].astype(F32)

    in_maps1 = []
    for core in range(NCORES):
        b, h = core // 2, core % 2
        rs = 1 + h * HS  # padded-coords start row (rows = interior -1..129)
        m = dict(common1)
        m["xh"] = np.ascontiguousarray(xpad[b, :, rs : rs + PH1, :]).astype(BF16)
        m["yh"] = np.ascontiguousarray(ypad[b, :, rs : rs + PH1, :]).astype(BF16)
        in_maps1.append(m)

    res1 = bass_utils.run_bass_kernel_spmd(k1, in_maps1, core_ids=list(range(NCORES)))

    # ---------- host combine: attention softmax -> P = w_proj @ blockdiag(A)
    pts = []
    vfull = np.zeros((B, C, H, W), BF16)
    for b in range(B):
        r0, r1 = res1.results[2 * b], res1.results[2 * b + 1]
        G128 = r0["qg"].astype(np.float64) + r1["qg"].astype(np.float64)
        G = G128[0:C, C:128]
        qss = np.diag(G128[0:C, 0:C])
        kss = np.diag(G128[C:128, C:128])
        nq = np.maximum(np.sqrt(qss), 1e-12)
        nk = np.maximum(np.sqrt(kss), 1e-12)
        A = np.zeros((C, C), np.float64)
        for hh in range(HEADS):
            sl = slice(hh * CH, (hh + 1) * CH)
            logits = temperature[hh, 0, 0] * (G[sl, sl] / np.outer(nq[sl], nk[sl]))
            e = np.exp(logits - logits.max(axis=-1, keepdims=True))
            A[sl, sl] = e / e.sum(axis=-1, keepdims=True)
        P = w_proj.astype(np.float64) @ A
        pts.append(np.ascontiguousarray(P.T).astype(BF16))
        vfull[b, :, 0:HS] = r0["vout"]
        vfull[b, :, HS:H] = r1["vout"]

    # ---------- launch 2: x1 = x + P v ; IEL (v2: fp8 DoubleRow kernel)
    vpad = np.zeros((B, C, H + 4, W), BF16)
    vpad[:, :, 2 : 2 + H, :] = vfull
    xpad16 = xpad.astype(BF16)

    w_in_p = np.zeros((NG * 128, C), F32)
    w_dw_p = np.zeros((NG * 128, 3, 3), F32)
    w12 = np.concatenate([w_dw1[:, 0], w_dw2[:, 0]], axis=0)  # [340,3,3]
    w12_p = np.zeros((NG * 128, 3, 3), F32)
    for s, ch in enumerate(PLACE340):
        if ch >= 0:
            w_in_p[s] = w_in[ch]
            w_dw_p[s] = w_dw[ch, 0]
            w12_p[s] = w12[ch]
    SC1 = float(2.0 ** SC1E)
    SC2 = float(2.0 ** SC2E)
    # fused-d weights: DR pairs (dy=-1,+1), DR singles, norm pair/single (dy=0)
    fdrp = np.zeros((128, NG, 2, 128), F32)
    fdrs = np.zeros((C, NG, 2, 128), F32)
    fnp = np.zeros((128, NG, 128), F32)
    fns = np.zeros((C, NG, 128), F32)
    for g in range(NG):
        sl = slice(g * 128, (g + 1) * 128)
        wi = w_in_p[sl]  # [128m, 64c]
        wd = w_dw_p[sl]  # [128m, 3, 3]
        for t, dy in enumerate((0, 2)):  # tap rows: dy=-1 -> 0, dy=+1 -> 2
            fdrp[0:C, g, t, :] = (wi * wd[:, dy, 0][:, None]).T  # dx=-1 half
            fdrp[C:128, g, t, :] = (wi * wd[:, dy, 2][:, None]).T  # dx=+1 half
            fdrs[:, g, t, :] = (wi * wd[:, dy, 1][:, None]).T
        fnp[0:C, g, :] = (wi * wd[:, 1, 0][:, None]).T
        fnp[C:128, g, :] = (wi * wd[:, 1, 2][:, None]).T
        fns[:, g, :] = (wi * wd[:, 1, 1][:, None]).T
    # dw12 diag weights
    wdr = np.zeros((128, NG, 3, 2, 128), F32)
    wn = np.zeros((128, NG, 3, 128), F32)
    for g in range(NG):
        sl = slice(g * 128, (g + 1) * 128)
        for dx in range(3):
            for t, dy in enumerate((0, 2)):
                wdr[np.arange(128), g, dx, t, np.arange(128)] = w12_p[sl, dy, dx]
            wn[np.arange(128), g, dx, np.arange(128)] = w12_p[sl, 1, dx]
    common2 = {
        "vsel": vsel.astype(BF16),
        "bsel": bsel.astype(BF16),
        "fdrp": (fdrp * SC1).astype(E4M3),
        "fdrs": (fdrs * SC1).astype(E4M3),
        "fnp": (fnp * SC1).astype(E4M3),
        "fns": (fns * SC1).astype(E4M3),
        "wdr": (wdr * SC2).astype(E4M3),
        "wn": (wn * SC2).astype(E4M3),
        "wouta": np.ascontiguousarray(w_out.T[0:128]).astype(BF16),
        "woutb": np.ascontiguousarray(w_out.T[128:170]).astype(BF16),
    }
    if affine:
        common2["gamv"] = ln_w[:, None].astype(F32)
        common2["betv"] = ln_b[:, None].astype(F32)

    in_maps2 = []
    for core in range(NCORES):
        b, h = core // 2, core % 2
        rs = h * HS  # padded-coords start row (halo-2 slab of 132 rows)
        m = dict(common2)
        m["xk"] = np.ascontiguousarray(xpad16[b, :, rs : rs + HS + 4, :])
        m["vk"] = np.ascontiguousarray(vpad[b, :, rs : rs + HS + 4, :])
        m["ptw"] = pts[b]
        in_maps2.append(m)

    res2 = bass_utils.run_bass_kernel_spmd(k2, in_maps2, core_ids=list(range(NCORES)))

    out = np.zeros((B, C, H, W), F32)
    for core in range(NCORES):
        b, h = core // 2, core % 2
        out[b, :, h * HS : (h + 1) * HS, :] = res2.results[core]["out"].astype(F32)
    return out

